# revision 1
# baseline (speedup 1.0000x reference)
"""Trainium2 Bass kernel for a single transformer decoder layer.

Reference semantics (B=64, T=200, E=512, H=8, D=64):
  x += SelfAttn(LN1(x))   (q,k row-masked by pred_mask, causal)
  x += CrossAttn(LN2(x))  (k from raw memory row-masked by src_mask,
                           v from LN2(x) (!), causal)
  x += FFN(LN3(x))        (512 -> 2048 -> relu -> 512)

Sharding: data-parallel over batch, 8 elems per NeuronCore, no collectives.

Layout strategy (per core, batch elems processed in PAIRS):
  - residual stream x kept NATURAL [t_chunk<=128, 512] in fp32
  - LN via bn_stats/bn_aggr + two fused scalar_tensor_tensor ops
  - activations transposed to [E, 2*T] pair tiles via PE is_transpose
    matmuls (keeps PE warm), DVE drains the PSUM
  - Q,K projected transposed [H*D, 2*T] with weight stationaries, N=400
  - scores computed TRANSPOSED  ST[s, t] = K Q^T  per head per elem,
    2 heads per PSUM bank; exp on ACT (no max subtraction -- scores are
    O(1)); causal mask applied post-exp via gpsimd.affine_select(fill=0)
  - matmul operands must sit at SBUF base partition 0 (row-group-64
    operands crash the device), so odd heads read DMA-shifted copies
  - softmax denominators via one-hot-column matmuls into [8,T] PSUM;
    1/d via reciprocal_approx_fast, broadcast to head halves by a
    one-hot matmul, multiplied into O^T on DVE
  - AV gives O transposed directly (lhsT = V natural slices)
  - biases enter PSUM via rank-1 (K=1) matmuls; FFN b1 rides the
    relu activation bias (per-partition in the transposed layout)
"""

import numpy as np
import ml_dtypes
from contextlib import ExitStack

import concourse.bass as bass
import concourse.bacc as bacc
import concourse.tile as tile
from concourse import mybir
from concourse.bass_utils import run_bass_kernel_spmd

B, T, E, H, Dh, F = 64, 200, 512, 8, 64, 2048
NCORES = 8
SCALE = float(E) ** -0.5
F32 = mybir.dt.float32
BF16 = mybir.dt.bfloat16
AL = mybir.AluOpType
AF = mybir.ActivationFunctionType
TCH = [(0, 128), (128, 72)]  # token chunks (t0, tc)
ECH = E // 128  # 4
FCH = F // 128  # 16
NPBF16 = ml_dtypes.bfloat16

_programs = {}


def _layernorm(nc, pools, x_c, tc, eps):
    """x_c: [tc,512] f32 natural -> (x-mu)*rsqrt(var+eps) as bf16.
    LN gamma is folded into the downstream weights host-side; beta enters
    via rank-1 bias matmuls."""
    st6 = pools["small"].tile([tc, 6], F32, name="st6")
    nc.vector.bn_stats(st6[:, :], x_c)
    mv = pools["small"].tile([tc, 2], F32, name="mv")
    nc.vector.bn_aggr(mv[:, :], st6[:, :])
    std = pools["small"].tile([tc, 1], F32, name="std")
    nc.scalar.activation(std[:, :], mv[:, 1:2], AF.Sqrt, bias=eps[0:tc, 0:1])
    rstd = pools["small"].tile([tc, 1], F32, name="rstd")
    nc.vector.reciprocal(rstd[:, :], std[:, :])
    nb = pools["small"].tile([tc, 1], F32, name="nb")
    nc.vector.tensor_scalar(nb[:, :], mv[:, 0:1], rstd[:, 0:1], -1.0,
                            op0=AL.mult, op1=AL.mult)
    h_c = pools["h"].tile([tc, E], BF16, name="h_c", tag="h_c", bufs=6)
    nc.scalar.activation(h_c[:, :], x_c, AF.Identity, scale=rstd[:, 0:1],
                         bias=nb[:, 0:1])
    return h_c


def _transpose_pair(nc, pools, h_cs_pair, ident):
    """h_cs_pair: list of 2 elems x 2 chunks of [tc,512] bf16 natural ->
    hT[ec] [128, 400] bf16 pair tiles via PE transposes."""
    hT = []
    for ec in range(ECH):
        t = pools["tT"].tile([128, 2 * T], BF16, name="hT", bufs=6)
        for el in range(2):
            for ci, (t0, tc) in enumerate(TCH):
                ps = pools["ps"].tile([128, tc], BF16, name="t_ps", tag="ps")
                nc.tensor.transpose(
                    ps[:, :], h_cs_pair[el][ci][0:tc, ec * 128:(ec + 1) * 128],
                    ident[0:tc, 0:tc])
                nc.vector.tensor_copy(t[:, el * T + t0:el * T + t0 + tc], ps[:, :])
        hT.append(t)
    return hT


def _project_qkT(nc, pools, w_sb, rhs_T, name, brow=None, mrow=None):
    """[128, 400] bf16 pair chunks of (W^T h)^T, plus base-partition-0
    copies of rows 64:128 (odd heads must read from partition 0).
    brow: [1,512] LN-beta@W row, added as a rank-1 term (masked by mrow)."""
    out, hi = [], []
    for oc in range(4):
        ps = pools["ps"].tile([128, 2 * T], F32, name=f"{name}_ps", tag="ps")
        for ec in range(ECH):
            nc.tensor.matmul(ps[:, :], w_sb[:, ec, oc * 128:(oc + 1) * 128],
                             rhs_T[ec][:, :], start=(ec == 0),
                             stop=(ec == 3 and brow is None))
        if brow is not None:
            nc.tensor.matmul(ps[:, :], brow[0:1, oc * 128:(oc + 1) * 128],
                             mrow[0:1, :], start=False, stop=True)
        qk = "q" if name.startswith("q") else "k"
        sb = pools["qkt"].tile([128, 2 * T], BF16, name=f"{name}_sb", tag=qk, bufs=5)
        nc.vector.tensor_copy(sb[:, :], ps[:, :])
        hb = pools["qkt"].tile([64, 2 * T], BF16, name=f"{name}_hi", tag="hi",
                               bufs=10)
        nc.sync.dma_start(hb[:, :], sb[64:128, :])
        out.append(sb)
        hi.append(hb)
    return out, hi


def _project_v(nc, pools, wv_sb, hT, off, name, brow=None, ones_row=None):
    """v natural [tc, 512] bf16 tiles for ONE elem (lhsT = hT pair slices)."""
    out = []
    for (t0, tc) in TCH:
        ps = pools["ps"].tile([tc, E], F32, name=f"{name}_ps", tag="ps")
        for ec in range(ECH):
            nc.tensor.matmul(ps[:, :], hT[ec][:, off + t0:off + t0 + tc],
                             wv_sb[:, ec, :], start=(ec == 0),
                             stop=(ec == 3 and brow is None))
        if brow is not None:
            nc.tensor.matmul(ps[:, :], ones_row[0:1, 0:tc], brow[0:1, :],
                             start=False, stop=True)
        sb = pools["v"].tile([tc, E], BF16, name=f"{name}_sb", tag="v", bufs=6)
        nc.scalar.copy(sb[:, :], ps[:, :])
        out.append(sb)
    return out


def _attention(nc, pools, qkt, v_sb, sel_sb, selB, wo_sb, bo_row, ones_row,
               x_cs, off):
    """Causal attention for ONE elem (token cols off:off+200 of the pair
    tiles) + output projection + bias + residual."""
    (qT_lo, qT_hi), (kT_lo, kT_hi) = qkt
    e0m, e1m = [], []
    # pass A: scores (transposed), exp, causal select; 2 heads per psum bank
    for oc in range(4):
        st0 = pools["ps"].tile([128, 2, 200], F32, name="st0", tag="ps")
        st1 = pools["ps"].tile([72, 2, 72], F32, name="st1", tag="ps")
        for hl in range(2):
            qh = (qT_lo, qT_hi)[hl][oc][0:64, off:off + 200]
            kh = (kT_lo, kT_hi)[hl][oc][0:64, off:off + 200]
            nc.tensor.matmul(st0[:, hl, :], kh[:, 0:128], qh)
            nc.tensor.matmul(st1[:, hl, :], kh[:, 128:200], qh[:, 128:200])
        e0 = pools["e0"].tile([128, 2, 200], BF16, name="e0", bufs=3)
        nc.scalar.activation(e0[:, :, :], st0[:, :, :], AF.Exp, scale=SCALE)
        e1 = pools["e1"].tile([72, 2, 72], BF16, name="e1", bufs=3)
        nc.scalar.activation(e1[:, :, :], st1[:, :, :], AF.Exp, scale=SCALE)
        # causal: keep where t - s >= 0 (iota = -p + t), else 0
        e0x = pools["e0"].tile([128, 2, 200], BF16, name="e0x", bufs=5)
        nc.gpsimd.affine_select(
            e0x[:, :, :], e0[:, :, :], pattern=[[0, 2], [1, 200]],
            compare_op=AL.is_ge, fill=0.0, base=0, channel_multiplier=-1)
        e1x = pools["e1"].tile([72, 2, 72], BF16, name="e1x", bufs=5)
        nc.gpsimd.affine_select(
            e1x[:, :, :], e1[:, :, :], pattern=[[0, 2], [1, 72]],
            compare_op=AL.is_ge, fill=0.0, base=0, channel_multiplier=-1)
        e0m.append(e0x)
        e1m.append(e1x)
    # pass B: denominators d[h, t] = sum_s exp -- one-hot stationaries
    dT = pools["ps"].tile([8, 200], F32, name="dT", tag="ps")
    for oc in range(4):
        for hl in range(2):
            h = 2 * oc + hl
            nc.tensor.matmul(dT[:, 0:200], sel_sb[0:128, h, :], e0m[oc][:, hl, :],
                             start=(h == 0), stop=False, skip_group_check=True)
            nc.tensor.matmul(dT[:, 128:200], sel_sb[0:72, h, :], e1m[oc][:, hl, :],
                             start=False, stop=(h == 7), skip_group_check=True)
    dt_sb = pools["small"].tile([8, 200], F32, name="dt_sb")
    nc.vector.tensor_copy(dt_sb[:, :], dT[:, :])
    dinvT = pools["small"].tile([8, 200], F32, name="dinvT")
    nc.vector.reciprocal_approx_fast(dinvT[:, :], dt_sb[:, :])
    # pass C: O^T = V^T @ E^T, normalized by 1/d broadcast to head halves
    oT_sb = []
    for oc in range(4):
        dbc_ps = pools["ps"].tile([128, 200], F32, name="dbc_ps", tag="ps")
        nc.tensor.matmul(dbc_ps[:, :], selB[0:8, oc, :], dinvT[:, :])
        dbc = pools["dbc"].tile([128, 200], F32, name="dbc")
        nc.vector.tensor_copy(dbc[:, :], dbc_ps[:, :])
        ot_ps = pools["ps"].tile([128, 200], F32, name="ot_ps", tag="ps")
        for hl in range(2):
            h = 2 * oc + hl
            hp = hl * 64
            nc.tensor.matmul(ot_ps[hp:hp + 64, 0:200],
                             v_sb[0][0:128, h * 64:(h + 1) * 64],
                             e0m[oc][:, hl, :], start=True, stop=False,
                             skip_group_check=True)
            nc.tensor.matmul(ot_ps[hp:hp + 64, 128:200],
                             v_sb[1][0:72, h * 64:(h + 1) * 64],
                             e1m[oc][:, hl, :], start=False, stop=True,
                             skip_group_check=True)
        ot = pools["ot"].tile([128, 200], BF16, name="ot", bufs=6)
        nc.vector.tensor_mul(ot[:, :], ot_ps[:, :], dbc[:, :])
        oT_sb.append(ot)
    # output projection (natural) + bias via rank-1 matmul + residual
    new_x = []
    for ci, (t0, tc) in enumerate(TCH):
        ps = pools["ps"].tile([tc, E], F32, name="proj_ps", tag="ps")
        for hc in range(4):
            nc.tensor.matmul(ps[:, :], oT_sb[hc][:, t0:t0 + tc],
                             wo_sb[:, hc, :], start=(hc == 0), stop=False)
        nc.tensor.matmul(ps[:, :], ones_row[0:1, 0:tc], bo_row[0:1, :],
                         start=False, stop=True)
        xn = pools["res"].tile([tc, E], F32, name="xn", tag="res")
        nc.vector.tensor_add(xn[:, :], ps[:, :], x_cs[ci])
        new_x.append(xn)
    return new_x


def _build(bpc, stages=3):
    nc = bacc.Bacc("TRN2", target_bir_lowering=False, debug=False,
                   enable_asserts=False, num_devices=NCORES)
    dram = {}

    def din(name, shape, dt):
        h = nc.dram_tensor(name, list(shape), dt, kind="ExternalInput")
        dram[name] = h
        return h

    x_d = din("x", (bpc, T, E), F32)
    mem_d = din("mem", (bpc, T, E), BF16)
    pm_d = din("pm", (bpc, T), BF16)
    sm_d = din("sm", (bpc, T), BF16)
    wq_sa_d = din("wq_sa", (E, E), BF16)
    wk_sa_d = din("wk_sa", (E, E), BF16)
    wv_sa_d = din("wv_sa", (E, E), BF16)
    wo_sa_d = din("wo_sa", (E, E), BF16)
    bo_sa_d = din("bo_sa", (1, E), BF16)
    wq_ca_d = din("wq_ca", (E, E), BF16)
    wk_ca_d = din("wk_ca", (E, E), BF16)
    wv_ca_d = din("wv_ca", (E, E), BF16)
    wo_ca_d = din("wo_ca", (E, E), BF16)
    bo_ca_d = din("bo_ca", (1, E), BF16)
    w1_d = din("w1", (E, F), BF16)
    b1_d = din("b1", (1, F), BF16)
    w2_d = din("w2", (F, E), BF16)
    b2_d = din("b2", (1, E), BF16)
    bq_sa_d = din("bq_sa", (1, E), BF16)
    bk_sa_d = din("bk_sa", (1, E), BF16)
    bv_sa_d = din("bv_sa", (1, E), BF16)
    bq_ca_d = din("bq_ca", (1, E), BF16)
    bv_ca_d = din("bv_ca", (1, E), BF16)
    out_d = nc.dram_tensor("out", [bpc, T, E], F32, kind="ExternalOutput")

    sel_np = np.zeros((128, 8, 8), dtype=NPBF16)
    for h in range(8):
        sel_np[:, h, h] = 1
    sel_d = nc.inline_tensor(sel_np, name="selc")
    ones_d = nc.inline_tensor(np.ones((1, E), dtype=NPBF16), name="onesc")
    selB_np = np.zeros((8, 4, 128), dtype=np.float32)
    for oc in range(4):
        selB_np[2 * oc, oc, 0:64] = 1
        selB_np[2 * oc + 1, oc, 64:128] = 1
    selB_d = nc.inline_tensor(selB_np, name="selBc")
    identb_d = nc.inline_tensor(np.eye(128, dtype=NPBF16), name="identbc")

    with tile.TileContext(nc) as tcx, ExitStack() as ctx:
        pools = {}

        def pool(name, bufs, space="SBUF"):
            pools[name] = ctx.enter_context(
                tcx.tile_pool(name=name, bufs=bufs, space=space))
            return pools[name]

        wpool = pool("w", 1)
        pool("small", 6)
        pool("lnt", 3)
        pool("h", 6)
        pool("tT", 5)
        pool("qkt", 5)
        pool("v", 5)
        pool("e0", 3)
        pool("e1", 3)
        pool("ot", 6)
        pool("dbc", 3)
        pool("res", 12)
        pool("rT", 17)
        pool("mrow", 3)
        pool("mbc", 5)
        pool("ps", 8, space="PSUM")

        def wtile(name, src, shape, rearr=None, dt=BF16, eng=None):
            t = wpool.tile(shape, dt, tag=name, bufs=1, name=name)
            ap = src[:] if rearr is None else src[:].rearrange(rearr, p=128)
            (eng or nc.sync).dma_start(t[...], ap)
            return t

        # SA weights first (sync queue) so pair 0 starts quickly; bulk
        # FFN/CA weights go on the scalar HWDGE queue in parallel
        identb = wtile("identb", identb_d, [128, 128])
        sel_sb = wtile("sel", sel_d, [128, 8, 8])
        selB = wtile("selB", selB_d, [8, 4, 128], dt=F32)
        ones_row = wtile("ones", ones_d, [1, E])
        wq_sa = wtile("wq_sa", wq_sa_d, [128, ECH, E], "(c p) n -> p c n")
        wk_sa = wtile("wk_sa", wk_sa_d, [128, ECH, E], "(c p) n -> p c n")
        wv_sa = wtile("wv_sa", wv_sa_d, [128, ECH, E], "(c p) n -> p c n")
        wo_sa = wtile("wo_sa", wo_sa_d, [128, ECH, E], "(c p) n -> p c n")
        bq_sa = wtile("bq_sa", bq_sa_d, [1, E])
        bk_sa = wtile("bk_sa", bk_sa_d, [1, E])
        bv_sa = wtile("bv_sa", bv_sa_d, [1, E])
        bo_sa = wtile("bo_sa", bo_sa_d, [1, E])
        wq_ca = wtile("wq_ca", wq_ca_d, [128, ECH, E], "(c p) n -> p c n",
                      eng=nc.scalar)
        wk_ca = wtile("wk_ca", wk_ca_d, [128, ECH, E], "(c p) n -> p c n",
                      eng=nc.scalar)
        wv_ca = wtile("wv_ca", wv_ca_d, [128, ECH, E], "(c p) n -> p c n",
                      eng=nc.scalar)
        wo_ca = wtile("wo_ca", wo_ca_d, [128, ECH, E], "(c p) n -> p c n",
                      eng=nc.scalar)
        bq_ca = wtile("bq_ca", bq_ca_d, [1, E], eng=nc.scalar)
        bv_ca = wtile("bv_ca", bv_ca_d, [1, E], eng=nc.scalar)
        bo_ca = wtile("bo_ca", bo_ca_d, [1, E], eng=nc.scalar)
        w1 = wtile("w1", w1_d, [128, ECH, F], "(c p) n -> p c n", eng=nc.scalar)
        w2 = wtile("w2", w2_d, [128, FCH, E], "(c p) n -> p c n", eng=nc.scalar)
        b2r = wtile("b2", b2_d, [1, E], eng=nc.scalar)
        # f_b1 (+ folded ln3_b @ w1) in column layout for the relu bias
        b1c = wpool.tile([128, FCH], F32, tag="b1c", bufs=1, name="b1c")
        b1cb = wpool.tile([128, FCH], BF16, tag="b1cb", bufs=1, name="b1cb")
        nc.scalar.dma_start(b1cb[...],
                            b1_d[:].rearrange("o (c p) -> p (o c)", p=128))
        nc.vector.tensor_copy(b1c[:, :], b1cb[:, :])
        eps = wpool.tile([128, 1], F32, tag="eps", bufs=1, name="eps")
        nc.gpsimd.memset(eps[:, :], 1e-5)

        for pr in range(bpc // 2):
            els = (2 * pr, 2 * pr + 1)
            # ---- load x and masks for both elems ----
            x_el = []
            pm2 = pools["mbc"].tile([128, 2 * T], BF16, name="pm2")
            sm2 = pools["mbc"].tile([128, 2 * T], BF16, name="sm2")
            pmrow2 = pools["mrow"].tile([1, 2 * T], BF16, name="pmrow2", bufs=2)
            ones2 = pools["mrow"].tile([1, 2 * T], BF16, name="ones2", bufs=2)
            nc.gpsimd.memset(ones2[:, :], 1.0)
            for el, e in enumerate(els):
                x_cs = []
                for (t0, tc) in TCH:
                    xt = pools["res"].tile([tc, E], F32, name="x_in", tag="res")
                    nc.sync.dma_start(xt[:, :], x_d[e, t0:t0 + tc, :])
                    x_cs.append(xt)
                x_el.append(x_cs)
                nc.sync.dma_start(pmrow2[0:1, el * T:(el + 1) * T],
                                  pm_d[e:e + 1, :])
                nc.gpsimd.partition_broadcast(pm2[:, el * T:(el + 1) * T],
                                              pmrow2[0:1, el * T:(el + 1) * T])
                sm_row = pools["mrow"].tile([1, T], BF16, name="sm_row", bufs=2)
                nc.sync.dma_start(sm_row[:, :], sm_d[e:e + 1, :])
                nc.gpsimd.partition_broadcast(sm2[:, el * T:(el + 1) * T],
                                              sm_row[:, :])

            # ======== self-attention ========
            h_pair = [[_layernorm(nc, pools, x_el[el][ci][:, :], tc, eps)
                       for ci, (t0, tc) in enumerate(TCH)] for el in range(2)]
            hT = _transpose_pair(nc, pools, h_pair, identb)
            hmT = []
            for ec in range(ECH):
                m = pools["tT"].tile([128, 2 * T], BF16, name="hmT", bufs=5)
                nc.vector.tensor_mul(m[:, :], hT[ec][:, :], pm2[:, :])
                hmT.append(m)
            qT = _project_qkT(nc, pools, wq_sa, hmT, "q_sa", bq_sa, pmrow2)
            kT = _project_qkT(nc, pools, wk_sa, hmT, "k_sa", bk_sa, pmrow2)
            for el in range(2):
                v_sb = _project_v(nc, pools, wv_sa, hT, el * T, "v_sa",
                                  bv_sa, ones_row)
                x_el[el] = _attention(nc, pools, (qT, kT), v_sb, sel_sb, selB,
                                      wo_sa, bo_sa, ones_row, x_el[el], el * T)
            if stages == 1:
                for el, e in enumerate(els):
                    for ci, (t0, tc) in enumerate(TCH):
                        nc.sync.dma_start(out_d[e, t0:t0 + tc, :],
                                          x_el[el][ci][:, :])
                continue

            # ======== cross-attention ========
            h_pair = [[_layernorm(nc, pools, x_el[el][ci][:, :], tc, eps)
                       for ci, (t0, tc) in enumerate(TCH)] for el in range(2)]
            h2T = _transpose_pair(nc, pools, h_pair, identb)
            m_pair = []
            for el, e in enumerate(els):
                m_cs = []
                for (t0, tc) in TCH:
                    mt = pools["h"].tile([tc, E], BF16, name="m_nat",
                                         tag="m_nat", bufs=6)
                    nc.sync.dma_start(mt[:, :], mem_d[e, t0:t0 + tc, :])
                    m_cs.append(mt)
                m_pair.append(m_cs)
            mT = _transpose_pair(nc, pools, m_pair, identb)
            memT = []
            for ec in range(ECH):
                mm = pools["tT"].tile([128, 2 * T], BF16, name="memTm", bufs=5)
                nc.vector.tensor_mul(mm[:, :], mT[ec][:, :], sm2[:, :])
                memT.append(mm)
            qT = _project_qkT(nc, pools, wq_ca, h2T, "q_ca", bq_ca, ones2)
            kT = _project_qkT(nc, pools, wk_ca, memT, "k_ca")
            for el in range(2):
                v_sb = _project_v(nc, pools, wv_ca, h2T, el * T, "v_ca",
                                  bv_ca, ones_row)
                x_el[el] = _attention(nc, pools, (qT, kT), v_sb, sel_sb, selB,
                                      wo_ca, bo_ca, ones_row, x_el[el], el * T)
            if stages == 2:
                for el, e in enumerate(els):
                    for ci, (t0, tc) in enumerate(TCH):
                        nc.sync.dma_start(out_d[e, t0:t0 + tc, :],
                                          x_el[el][ci][:, :])
                continue

            # ======== feed-forward ========
            h_pair = [[_layernorm(nc, pools, x_el[el][ci][:, :], tc, eps)
                       for ci, (t0, tc) in enumerate(TCH)] for el in range(2)]
            h3T = _transpose_pair(nc, pools, h_pair, identb)
            rT = []
            for fc in range(FCH):
                zps = pools["ps"].tile([128, 2 * T], F32, name="z_ps",
                                          tag="ps")
                for ec in range(ECH):
                    nc.tensor.matmul(zps[:, :],
                                     w1[:, ec, fc * 128:(fc + 1) * 128],
                                     h3T[ec][:, :], start=(ec == 0),
                                     stop=(ec == 3))
                r = pools["rT"].tile([128, 2 * T], BF16, name="r")
                nc.scalar.activation(r[:, :], zps[:, :], AF.Relu,
                                     bias=b1c[:, fc:fc + 1])
                rT.append(r)
            for el, e in enumerate(els):
                for ci, (t0, tc) in enumerate(TCH):
                    yps = pools["ps"].tile([tc, E], F32, name="y_ps",
                                                tag="ps")
                    for fc in range(FCH):
                        nc.tensor.matmul(yps[:, :],
                                         rT[fc][:, el * T + t0:el * T + t0 + tc],
                                         w2[:, fc, :], start=(fc == 0),
                                         stop=False)
                    nc.tensor.matmul(yps[:, :], ones_row[0:1, 0:tc],
                                     b2r[0:1, :], start=False, stop=True)
                    yout = pools["res"].tile([tc, E], F32, name="yout",
                                             tag="res")
                    nc.vector.tensor_add(yout[:, :], yps[:, :],
                                         x_el[el][ci][:, :])
                    nc.sync.dma_start(out_d[e, t0:t0 + tc, :], yout[:, :])

    nc.compile()
    return nc


def _host_prep(inputs, bpc, core):
    """Build the in_map for one core."""
    s = slice(core * bpc, (core + 1) * bpc)

    def rearr(w, g=None):  # (H, E, D) -> [E, H*D], optionally row-scaled
        m = np.transpose(np.asarray(w, np.float32), (1, 0, 2)).reshape(E, E)
        if g is not None:
            m = m * np.asarray(g, np.float32)[:, None]
        return np.ascontiguousarray(m).astype(NPBF16)

    def b16(a):
        return np.ascontiguousarray(np.asarray(a, np.float32)).astype(NPBF16)

    def f32c(a):
        return np.ascontiguousarray(np.asarray(a, np.float32))

    g1 = np.asarray(inputs["ln1_g"], np.float32)
    b1n = np.asarray(inputs["ln1_b"], np.float32)
    g2 = np.asarray(inputs["ln2_g"], np.float32)
    b2n = np.asarray(inputs["ln2_b"], np.float32)
    g3 = np.asarray(inputs["ln3_g"], np.float32)
    b3n = np.asarray(inputs["ln3_b"], np.float32)

    def wr(w):  # raw rearranged fp32 (for beta @ W rows)
        return np.transpose(np.asarray(w, np.float32), (1, 0, 2)).reshape(E, E)

    return {
        "x": f32c(inputs["idx"][s]),
        "mem": b16(inputs["memory"][s]),
        "pm": b16(inputs["pred_mask"][s] != 0),
        "sm": b16(inputs["src_mask"][s] != 0),
        "wq_sa": rearr(inputs["sa_wq"], g1), "wk_sa": rearr(inputs["sa_wk"], g1),
        "wv_sa": rearr(inputs["sa_wv"], g1),
        "wo_sa": b16(inputs["sa_wo"]), "bo_sa": b16(inputs["sa_bo"]).reshape(1, E),
        "bq_sa": b16(b1n @ wr(inputs["sa_wq"])).reshape(1, E),
        "bk_sa": b16(b1n @ wr(inputs["sa_wk"])).reshape(1, E),
        "bv_sa": b16(b1n @ wr(inputs["sa_wv"])).reshape(1, E),
        "wq_ca": rearr(inputs["ca_wq"], g2), "wk_ca": rearr(inputs["ca_wk"]),
        "wv_ca": rearr(inputs["ca_wv"], g2),
        "wo_ca": b16(inputs["ca_wo"]), "bo_ca": b16(inputs["ca_bo"]).reshape(1, E),
        "bq_ca": b16(b2n @ wr(inputs["ca_wq"])).reshape(1, E),
        "bv_ca": b16(b2n @ wr(inputs["ca_wv"])).reshape(1, E),
        "w1": b16(np.asarray(inputs["f_w1"], np.float32)
                  * g3[:, None]),
        "b1": b16(np.asarray(inputs["f_b1"], np.float32)
                  + b3n @ np.asarray(inputs["f_w1"], np.float32)).reshape(1, F),
        "w2": b16(inputs["f_w2"]), "b2": b16(inputs["f_b2"]).reshape(1, E),
    }


def get_program(bpc):
    if bpc not in _programs:
        _programs[bpc] = _build(bpc)
    return _programs[bpc]


def kernel(**inputs) -> np.ndarray:
    bpc = B // NCORES
    nc = get_program(bpc)
    in_maps = [_host_prep(inputs, bpc, c) for c in range(NCORES)]
    res = run_bass_kernel_spmd(nc, in_maps, core_ids=list(range(NCORES)))
    out = np.concatenate([res.results[c]["out"] for c in range(NCORES)], axis=0)
    return out.astype(np.float32)



# revision 9
# speedup vs baseline: 1.2926x; 1.2926x over previous
"""Trainium2 Bass kernel for a single transformer decoder layer.

Reference semantics (B=64, T=200, E=512, H=8, D=64):
  x += SelfAttn(LN1(x))   (q,k row-masked by pred_mask, causal)
  x += CrossAttn(LN2(x))  (k from raw memory row-masked by src_mask,
                           v from LN2(x) (!), causal)
  x += FFN(LN3(x))        (512 -> 2048 -> relu -> 512)

Sharding: data-parallel over batch, 8 elems per NeuronCore, no collectives.

Layout strategy (per core, batch elems processed in PAIRS):
  - residual stream x kept NATURAL [t_chunk<=128, 512] in fp32
  - LN via bn_stats/bn_aggr + two fused scalar_tensor_tensor ops
  - activations transposed to [E, 2*T] pair tiles via PE is_transpose
    matmuls (keeps PE warm), DVE drains the PSUM
  - Q,K projected transposed [H*D, 2*T] with weight stationaries, N=400
  - scores computed TRANSPOSED  ST[s, t] = K Q^T  per head per elem,
    2 heads per PSUM bank; exp on ACT (no max subtraction -- scores are
    O(1)); causal mask applied post-exp via gpsimd.affine_select(fill=0)
  - matmul operands must sit at SBUF base partition 0 (row-group-64
    operands crash the device), so odd heads read DMA-shifted copies
  - softmax denominators via one-hot-column matmuls into [8,T] PSUM;
    1/d via reciprocal_approx_fast, broadcast to head halves by a
    one-hot matmul, multiplied into O^T on DVE
  - AV gives O transposed directly (lhsT = V natural slices)
  - biases enter PSUM via rank-1 (K=1) matmuls; FFN b1 rides the
    relu activation bias (per-partition in the transposed layout)
"""

import numpy as np
import ml_dtypes
from contextlib import ExitStack

import concourse.bass as bass
import concourse.bacc as bacc
import concourse.tile as tile
from concourse import mybir
from concourse.bass_utils import run_bass_kernel_spmd

B, T, E, H, Dh, F = 64, 200, 512, 8, 64, 2048
NCORES = 8
SCALE = float(E) ** -0.5
F32 = mybir.dt.float32
BF16 = mybir.dt.bfloat16
AL = mybir.AluOpType
AF = mybir.ActivationFunctionType
TCH = [(0, 128), (128, 72)]  # token chunks (t0, tc)
ECH = E // 128  # 4
FCH = F // 128  # 16
NPBF16 = ml_dtypes.bfloat16

_programs = {}


def _layernorm(nc, pools, x_c, tc, eps):
    """x_c: [tc,512] f32 natural -> (x-mu)*rsqrt(var+eps) as bf16.
    LN gamma is folded into the downstream weights host-side; beta enters
    via rank-1 bias matmuls."""
    st6 = pools["small"].tile([tc, 6], F32, name="st6")
    nc.vector.bn_stats(st6[:, :], x_c)
    mv = pools["small"].tile([tc, 2], F32, name="mv")
    nc.vector.bn_aggr(mv[:, :], st6[:, :])
    std = pools["small"].tile([tc, 1], F32, name="std")
    nc.scalar.activation(std[:, :], mv[:, 1:2], AF.Sqrt, bias=eps[0:tc, 0:1])
    rstd = pools["small"].tile([tc, 1], F32, name="rstd")
    nc.vector.reciprocal(rstd[:, :], std[:, :])
    nb = pools["small"].tile([tc, 1], F32, name="nb")
    nc.vector.tensor_scalar(nb[:, :], mv[:, 0:1], rstd[:, 0:1], -1.0,
                            op0=AL.mult, op1=AL.mult)
    h_c = pools["h"].tile([tc, E], BF16, name="h_c", tag="h_c", bufs=6)
    nc.scalar.activation(h_c[:, :], x_c, AF.Identity, scale=rstd[:, 0:1],
                         bias=nb[:, 0:1])
    return h_c


def _transpose_pair(nc, pools, h_cs_pair, ident):
    """h_cs_pair: list of 2 elems x 2 chunks of [tc,512] bf16 natural ->
    hT[ec] [128, 400] bf16 pair tiles via PE transposes."""
    hT = []
    for ec in range(ECH):
        t = pools["tT"].tile([128, 2 * T], BF16, name="hT", bufs=14)
        for el in range(2):
            for ci, (t0, tc) in enumerate(TCH):
                ps = pools["ps"].tile([128, tc], BF16, name="t_ps", tag="ps")
                nc.tensor.transpose(
                    ps[:, :], h_cs_pair[el][ci][0:tc, ec * 128:(ec + 1) * 128],
                    ident[0:tc, 0:tc])
                nc.vector.tensor_copy(t[:, el * T + t0:el * T + t0 + tc], ps[:, :])
        hT.append(t)
    return hT


def _project_qkT(nc, pools, w_sb, rhs_T, name, bcol=None, mask=None):
    """[128, 400] bf16 pair chunks of (W^T h)^T, plus base-partition-0
    copies of rows 64:128 (odd heads must read from partition 0).
    bcol: [128, 4] per-partition bias columns (LN-beta@W, transposed),
    fused into the PSUM drain; mask: [128, 2T] token mask multiplied in."""
    out, hi = [], []
    for oc in range(4):
        ps = pools["ps"].tile([128, 2 * T], F32, name=f"{name}_ps", tag="ps")
        for ec in range(ECH):
            nc.tensor.matmul(ps[:, :], w_sb[:, ec, oc * 128:(oc + 1) * 128],
                             rhs_T[ec][:, :], start=(ec == 0),
                             stop=(ec == 3))
        qk = "q" if name.startswith("q") else "k"
        sb = pools["qkt"].tile([128, 2 * T], BF16, name=f"{name}_sb", tag=qk, bufs=5)
        if bcol is not None and mask is not None:
            nc.vector.scalar_tensor_tensor(sb[:, :], ps[:, :],
                                           bcol[:, oc:oc + 1], mask[:, :],
                                           op0=AL.add, op1=AL.mult)
        elif bcol is not None:
            nc.vector.tensor_scalar(sb[:, :], ps[:, :], bcol[:, oc:oc + 1],
                                    None, op0=AL.add)
        elif mask is not None:
            nc.vector.tensor_mul(sb[:, :], ps[:, :], mask[:, :])
        else:
            nc.vector.tensor_copy(sb[:, :], ps[:, :])
        hb = pools["qkt"].tile([64, 2 * T], BF16, name=f"{name}_hi", tag="hi",
                               bufs=10)
        nc.sync.dma_start(hb[:, :], sb[64:128, :])
        out.append(sb)
        hi.append(hb)
    return out, hi


def _project_v(nc, pools, wv_sb, hT, off, name):
    """v natural [tc, 512] bf16 tiles for ONE elem (lhsT = hT pair slices).
    The LN-beta@Wv bias is folded into the output-projection bias host-side
    (softmax rows sum to 1, so a constant v offset passes straight through)."""
    out = []
    for (t0, tc) in TCH:
        ps = pools["ps"].tile([tc, E], F32, name=f"{name}_ps", tag="ps")
        for ec in range(ECH):
            nc.tensor.matmul(ps[:, :], hT[ec][:, off + t0:off + t0 + tc],
                             wv_sb[:, ec, :], start=(ec == 0),
                             stop=(ec == 3))
        sb = pools["v"].tile([tc, E], BF16, name=f"{name}_sb", tag="v", bufs=6)
        nc.scalar.copy(sb[:, :], ps[:, :])
        out.append(sb)
    return out


def _attention(nc, pools, qkt, v_sb, ones64, wo_sb, bo_row, ones_row,
               x_cs, off):
    """Causal attention for ONE elem (token cols off:off+200 of the pair
    tiles) + output projection + bias + residual."""
    (qT_lo, qT_hi), (kT_lo, kT_hi) = qkt
    e0m, e1m = [], []
    # pass A: scores (transposed), exp, causal select; 2 heads per psum bank
    for oc in range(4):
        st0 = pools["ps"].tile([128, 2, 200], F32, name="st0", tag="ps")
        st1 = pools["ps"].tile([72, 2, 72], F32, name="st1", tag="ps")
        for hl in range(2):
            qh = (qT_lo, qT_hi)[hl][oc][0:64, off:off + 200]
            kh = (kT_lo, kT_hi)[hl][oc][0:64, off:off + 200]
            nc.tensor.matmul(st0[:, hl, :], kh[:, 0:128], qh)
            nc.tensor.matmul(st1[:, hl, :], kh[:, 128:200], qh[:, 128:200])
        e0 = pools["e0"].tile([128, 2, 200], BF16, name="e0", bufs=3)
        nc.scalar.activation(e0[:, :, :], st0[:, :, :], AF.Exp, scale=SCALE)
        e1 = pools["e1"].tile([72, 2, 72], BF16, name="e1", bufs=3)
        nc.scalar.activation(e1[:, :, :], st1[:, :, :], AF.Exp, scale=SCALE)
        # causal: keep where t - s >= 0 (iota = -p + t), else 0
        e0x = pools["e0"].tile([128, 2, 200], BF16, name="e0x", bufs=5)
        nc.gpsimd.affine_select(
            e0x[:, :, :], e0[:, :, :], pattern=[[0, 2], [1, 200]],
            compare_op=AL.is_ge, fill=0.0, base=0, channel_multiplier=-1)
        e1x = pools["e1"].tile([72, 2, 72], BF16, name="e1x", bufs=5)
        nc.gpsimd.affine_select(
            e1x[:, :, :], e1[:, :, :], pattern=[[0, 2], [1, 72]],
            compare_op=AL.is_ge, fill=0.0, base=0, channel_multiplier=-1)
        e0m.append(e0x)
        e1m.append(e1x)
    # pass B+C: per oc, broadcast-summed denominators via an all-ones
    # stationary (every output row = sum_s e[s,t], landing on the head's
    # partition range directly), reciprocal in-place, then O^T = V^T @ E^T
    oT_sb = []
    for oc in range(4):
        dbc_ps = pools["ps"].tile([128, 200], F32, name="dbc_ps", tag="ps")
        nc.tensor.matmul(dbc_ps[0:64, 0:200], ones64[0:128, :],
                         e0m[oc][:, 0, :], start=True, stop=False,
                         skip_group_check=True)
        nc.tensor.matmul(dbc_ps[64:128, 0:200], ones64[0:128, :],
                         e0m[oc][:, 1, :], start=True, stop=False,
                         skip_group_check=True)
        nc.tensor.matmul(dbc_ps[0:64, 128:200], ones64[0:72, :],
                         e1m[oc][:, 0, :], start=False, stop=False,
                         skip_group_check=True)
        nc.tensor.matmul(dbc_ps[64:128, 128:200], ones64[0:72, :],
                         e1m[oc][:, 1, :], start=False, stop=True,
                         skip_group_check=True)
        dbc = pools["dbc"].tile([128, 200], F32, name="dbc")
        nc.vector.reciprocal_approx_fast(dbc[:, :], dbc_ps[:, :])
        ot_ps = pools["ps"].tile([128, 200], F32, name="ot_ps", tag="ps")
        for hl in range(2):
            h = 2 * oc + hl
            hp = hl * 64
            nc.tensor.matmul(ot_ps[hp:hp + 64, 0:200],
                             v_sb[0][0:128, h * 64:(h + 1) * 64],
                             e0m[oc][:, hl, :], start=True, stop=False,
                             skip_group_check=True)
            nc.tensor.matmul(ot_ps[hp:hp + 64, 128:200],
                             v_sb[1][0:72, h * 64:(h + 1) * 64],
                             e1m[oc][:, hl, :], start=False, stop=True,
                             skip_group_check=True)
        ot = pools["ot"].tile([128, 200], BF16, name="ot", bufs=6)
        nc.vector.tensor_mul(ot[:, :], ot_ps[:, :], dbc[:, :])
        oT_sb.append(ot)
    # output projection (natural) + bias via rank-1 matmul + residual
    new_x = []
    for ci, (t0, tc) in enumerate(TCH):
        ps = pools["ps"].tile([tc, E], F32, name="proj_ps", tag="ps")
        for hc in range(4):
            nc.tensor.matmul(ps[:, :], oT_sb[hc][:, t0:t0 + tc],
                             wo_sb[:, hc, :], start=(hc == 0), stop=False)
        nc.tensor.matmul(ps[:, :], ones_row[0:1, 0:tc], bo_row[0:1, :],
                         start=False, stop=True)
        xn = pools["res"].tile([tc, E], F32, name="xn", tag="res")
        nc.vector.tensor_add(xn[:, :], ps[:, :], x_cs[ci])
        new_x.append(xn)
    return new_x


def _build(bpc, stages=3):
    nc = bacc.Bacc("TRN2", target_bir_lowering=False, debug=False,
                   enable_asserts=False, num_devices=NCORES)
    dram = {}

    def din(name, shape, dt):
        h = nc.dram_tensor(name, list(shape), dt, kind="ExternalInput")
        dram[name] = h
        return h

    x_d = din("x", (bpc, T, E), F32)
    mem_d = din("mem", (bpc, T, E), BF16)
    pm_d = din("pm", (bpc, T), BF16)
    sm_d = din("sm", (bpc, T), BF16)
    wq_sa_d = din("wq_sa", (E, E), BF16)
    wk_sa_d = din("wk_sa", (E, E), BF16)
    wv_sa_d = din("wv_sa", (E, E), BF16)
    wo_sa_d = din("wo_sa", (E, E), BF16)
    bo_sa_d = din("bo_sa", (1, E), BF16)
    wq_ca_d = din("wq_ca", (E, E), BF16)
    wk_ca_d = din("wk_ca", (E, E), BF16)
    wv_ca_d = din("wv_ca", (E, E), BF16)
    wo_ca_d = din("wo_ca", (E, E), BF16)
    bo_ca_d = din("bo_ca", (1, E), BF16)
    w1_d = din("w1", (E, F), BF16)
    b1_d = din("b1", (1, F), BF16)
    w2_d = din("w2", (F, E), BF16)
    b2_d = din("b2", (1, E), BF16)
    bq_sa_d = din("bq_sa", (128, 4), F32)
    bk_sa_d = din("bk_sa", (128, 4), F32)
    bq_ca_d = din("bq_ca", (128, 4), F32)
    out_d = nc.dram_tensor("out", [bpc, T, E], F32, kind="ExternalOutput")

    ones_d = nc.inline_tensor(np.ones((1, E), dtype=NPBF16), name="onesc")
    ones64_d = nc.inline_tensor(np.ones((128, 64), dtype=NPBF16), name="ones64c")
    identb_d = nc.inline_tensor(np.eye(128, dtype=NPBF16), name="identbc")

    with tile.TileContext(nc) as tcx, ExitStack() as ctx:
        pools = {}

        def pool(name, bufs, space="SBUF"):
            pools[name] = ctx.enter_context(
                tcx.tile_pool(name=name, bufs=bufs, space=space))
            return pools[name]

        wpool = pool("w", 1)
        pool("small", 6)
        pool("lnt", 3)
        pool("h", 6)
        pool("tT", 5)
        pool("qkt", 5)
        pool("v", 5)
        pool("e0", 3)
        pool("e1", 3)
        pool("ot", 6)
        pool("dbc", 3)
        pool("res", 12)
        pool("rT", 17)
        pool("mrow", 3)
        pool("mbc", 5)
        pool("ps", 8, space="PSUM")

        def wtile(name, src, shape, rearr=None, dt=BF16, eng=None):
            t = wpool.tile(shape, dt, tag=name, bufs=1, name=name)
            ap = src[:] if rearr is None else src[:].rearrange(rearr, p=128)
            (eng or nc.sync).dma_start(t[...], ap)
            return t

        # SA weights first (sync queue) so pair 0 starts quickly; bulk
        # FFN/CA weights go on the scalar HWDGE queue in parallel
        identb = wtile("identb", identb_d, [128, 128])
        ones64 = wtile("ones64", ones64_d, [128, 64])
        ones_row = wtile("ones", ones_d, [1, E])
        wq_sa = wtile("wq_sa", wq_sa_d, [128, ECH, E], "(c p) n -> p c n")
        wk_sa = wtile("wk_sa", wk_sa_d, [128, ECH, E], "(c p) n -> p c n")
        wv_sa = wtile("wv_sa", wv_sa_d, [128, ECH, E], "(c p) n -> p c n")
        wo_sa = wtile("wo_sa", wo_sa_d, [128, ECH, E], "(c p) n -> p c n")
        bq_sa = wtile("bq_sa", bq_sa_d, [128, 4], dt=F32)
        bk_sa = wtile("bk_sa", bk_sa_d, [128, 4], dt=F32)
        bo_sa = wtile("bo_sa", bo_sa_d, [1, E])
        wq_ca = wtile("wq_ca", wq_ca_d, [128, ECH, E], "(c p) n -> p c n",
                      eng=nc.scalar)
        wk_ca = wtile("wk_ca", wk_ca_d, [128, ECH, E], "(c p) n -> p c n",
                      eng=nc.scalar)
        wv_ca = wtile("wv_ca", wv_ca_d, [128, ECH, E], "(c p) n -> p c n",
                      eng=nc.scalar)
        wo_ca = wtile("wo_ca", wo_ca_d, [128, ECH, E], "(c p) n -> p c n",
                      eng=nc.scalar)
        bq_ca = wtile("bq_ca", bq_ca_d, [128, 4], dt=F32, eng=nc.scalar)
        bo_ca = wtile("bo_ca", bo_ca_d, [1, E], eng=nc.scalar)
        w1 = wtile("w1", w1_d, [128, ECH, F], "(c p) n -> p c n", eng=nc.scalar)
        w2 = wtile("w2", w2_d, [128, FCH, E], "(c p) n -> p c n", eng=nc.scalar)
        b2r = wtile("b2", b2_d, [1, E], eng=nc.scalar)
        # f_b1 (+ folded ln3_b @ w1) in column layout for the relu bias
        b1c = wpool.tile([128, FCH], F32, tag="b1c", bufs=1, name="b1c")
        b1cb = wpool.tile([128, FCH], BF16, tag="b1cb", bufs=1, name="b1cb")
        nc.scalar.dma_start(b1cb[...],
                            b1_d[:].rearrange("o (c p) -> p (o c)", p=128))
        nc.vector.tensor_copy(b1c[:, :], b1cb[:, :])
        eps = wpool.tile([128, 1], F32, tag="eps", bufs=1, name="eps")
        nc.gpsimd.memset(eps[:, :], 1e-5)

        for pr in range(bpc // 2):
            els = (2 * pr, 2 * pr + 1)
            # ---- load x and masks for both elems ----
            x_el = []
            pm2 = pools["mbc"].tile([128, 2 * T], BF16, name="pm2")
            sm2 = pools["mbc"].tile([128, 2 * T], BF16, name="sm2")
            for el, e in enumerate(els):
                x_cs = []
                for (t0, tc) in TCH:
                    xt = pools["res"].tile([tc, E], F32, name="x_in", tag="res")
                    nc.sync.dma_start(xt[:, :], x_d[e, t0:t0 + tc, :])
                    x_cs.append(xt)
                x_el.append(x_cs)
                pm_row = pools["mrow"].tile([1, T], BF16, name="pm_row", bufs=2)
                nc.sync.dma_start(pm_row[:, :], pm_d[e:e + 1, :])
                nc.gpsimd.partition_broadcast(pm2[:, el * T:(el + 1) * T],
                                              pm_row[:, :])
                sm_row = pools["mrow"].tile([1, T], BF16, name="sm_row", bufs=2)
                nc.sync.dma_start(sm_row[:, :], sm_d[e:e + 1, :])
                nc.gpsimd.partition_broadcast(sm2[:, el * T:(el + 1) * T],
                                              sm_row[:, :])

            # ======== self-attention ========
            h_pair = [[_layernorm(nc, pools, x_el[el][ci][:, :], tc, eps)
                       for ci, (t0, tc) in enumerate(TCH)] for el in range(2)]
            hT = _transpose_pair(nc, pools, h_pair, identb)
            qT = _project_qkT(nc, pools, wq_sa, hT, "q_sa", bq_sa, pm2)
            kT = _project_qkT(nc, pools, wk_sa, hT, "k_sa", bk_sa, pm2)
            for el in range(2):
                v_sb = _project_v(nc, pools, wv_sa, hT, el * T, "v_sa")
                x_el[el] = _attention(nc, pools, (qT, kT), v_sb, ones64,
                                      wo_sa, bo_sa, ones_row, x_el[el], el * T)
            if stages == 1:
                for el, e in enumerate(els):
                    for ci, (t0, tc) in enumerate(TCH):
                        nc.sync.dma_start(out_d[e, t0:t0 + tc, :],
                                          x_el[el][ci][:, :])
                continue

            # ======== cross-attention ========
            h_pair = [[_layernorm(nc, pools, x_el[el][ci][:, :], tc, eps)
                       for ci, (t0, tc) in enumerate(TCH)] for el in range(2)]
            h2T = _transpose_pair(nc, pools, h_pair, identb)
            m_pair = []
            for el, e in enumerate(els):
                m_cs = []
                for (t0, tc) in TCH:
                    mt = pools["h"].tile([tc, E], BF16, name="m_nat",
                                         tag="m_nat", bufs=6)
                    nc.sync.dma_start(mt[:, :], mem_d[e, t0:t0 + tc, :])
                    m_cs.append(mt)
                m_pair.append(m_cs)
            mT = _transpose_pair(nc, pools, m_pair, identb)
            qT = _project_qkT(nc, pools, wq_ca, h2T, "q_ca", bq_ca)
            kT = _project_qkT(nc, pools, wk_ca, mT, "k_ca", mask=sm2)
            for el in range(2):
                v_sb = _project_v(nc, pools, wv_ca, h2T, el * T, "v_ca")
                x_el[el] = _attention(nc, pools, (qT, kT), v_sb, ones64,
                                      wo_ca, bo_ca, ones_row, x_el[el], el * T)
            if stages == 2:
                for el, e in enumerate(els):
                    for ci, (t0, tc) in enumerate(TCH):
                        nc.sync.dma_start(out_d[e, t0:t0 + tc, :],
                                          x_el[el][ci][:, :])
                continue

            # ======== feed-forward ========
            h_pair = [[_layernorm(nc, pools, x_el[el][ci][:, :], tc, eps)
                       for ci, (t0, tc) in enumerate(TCH)] for el in range(2)]
            h3T = _transpose_pair(nc, pools, h_pair, identb)
            rT = []
            for fc in range(FCH):
                zps = pools["ps"].tile([128, 2 * T], F32, name="z_ps",
                                          tag="ps")
                for ec in range(ECH):
                    nc.tensor.matmul(zps[:, :],
                                     w1[:, ec, fc * 128:(fc + 1) * 128],
                                     h3T[ec][:, :], start=(ec == 0),
                                     stop=(ec == 3))
                r = pools["rT"].tile([128, 2 * T], BF16, name="r")
                nc.scalar.activation(r[:, :], zps[:, :], AF.Relu,
                                     bias=b1c[:, fc:fc + 1])
                rT.append(r)
            for el, e in enumerate(els):
                for ci, (t0, tc) in enumerate(TCH):
                    yps = pools["ps"].tile([tc, E], F32, name="y_ps",
                                                tag="ps")
                    for fc in range(FCH):
                        nc.tensor.matmul(yps[:, :],
                                         rT[fc][:, el * T + t0:el * T + t0 + tc],
                                         w2[:, fc, :], start=(fc == 0),
                                         stop=False)
                    nc.tensor.matmul(yps[:, :], ones_row[0:1, 0:tc],
                                     b2r[0:1, :], start=False, stop=True)
                    yout = pools["res"].tile([tc, E], F32, name="yout",
                                             tag="res")
                    nc.vector.tensor_add(yout[:, :], yps[:, :],
                                         x_el[el][ci][:, :])
                    nc.sync.dma_start(out_d[e, t0:t0 + tc, :], yout[:, :])

    nc.compile()
    return nc


def _host_prep(inputs, bpc, core):
    """Build the in_map for one core."""
    s = slice(core * bpc, (core + 1) * bpc)

    def rearr(w, g=None):  # (H, E, D) -> [E, H*D], optionally row-scaled
        m = np.transpose(np.asarray(w, np.float32), (1, 0, 2)).reshape(E, E)
        if g is not None:
            m = m * np.asarray(g, np.float32)[:, None]
        return np.ascontiguousarray(m).astype(NPBF16)

    def b16(a):
        return np.ascontiguousarray(np.asarray(a, np.float32)).astype(NPBF16)

    def f32c(a):
        return np.ascontiguousarray(np.asarray(a, np.float32))

    g1 = np.asarray(inputs["ln1_g"], np.float32)
    b1n = np.asarray(inputs["ln1_b"], np.float32)
    g2 = np.asarray(inputs["ln2_g"], np.float32)
    b2n = np.asarray(inputs["ln2_b"], np.float32)
    g3 = np.asarray(inputs["ln3_g"], np.float32)
    b3n = np.asarray(inputs["ln3_b"], np.float32)

    def wr(w):  # raw rearranged fp32 (for beta @ W rows)
        return np.transpose(np.asarray(w, np.float32), (1, 0, 2)).reshape(E, E)

    def bcolT(b):  # [E] bias row -> [128, 4] per-partition columns per oc
        return np.ascontiguousarray(
            np.asarray(b, np.float32).reshape(4, 128).T)

    wo_sa_f = np.asarray(inputs["sa_wo"], np.float32)
    wo_ca_f = np.asarray(inputs["ca_wo"], np.float32)
    bv_sa = b1n @ wr(inputs["sa_wv"])
    bv_ca = b2n @ wr(inputs["ca_wv"])

    return {
        "x": f32c(inputs["idx"][s]),
        "mem": b16(inputs["memory"][s]),
        "pm": b16(inputs["pred_mask"][s] != 0),
        "sm": b16(inputs["src_mask"][s] != 0),
        "wq_sa": rearr(inputs["sa_wq"], g1), "wk_sa": rearr(inputs["sa_wk"], g1),
        "wv_sa": rearr(inputs["sa_wv"], g1),
        "wo_sa": b16(inputs["sa_wo"]),
        "bo_sa": b16(np.asarray(inputs["sa_bo"], np.float32)
                     + bv_sa @ wo_sa_f).reshape(1, E),
        "bq_sa": bcolT(b1n @ wr(inputs["sa_wq"])),
        "bk_sa": bcolT(b1n @ wr(inputs["sa_wk"])),
        "wq_ca": rearr(inputs["ca_wq"], g2), "wk_ca": rearr(inputs["ca_wk"]),
        "wv_ca": rearr(inputs["ca_wv"], g2),
        "wo_ca": b16(inputs["ca_wo"]),
        "bo_ca": b16(np.asarray(inputs["ca_bo"], np.float32)
                     + bv_ca @ wo_ca_f).reshape(1, E),
        "bq_ca": bcolT(b2n @ wr(inputs["ca_wq"])),
        "w1": b16(np.asarray(inputs["f_w1"], np.float32)
                  * g3[:, None]),
        "b1": b16(np.asarray(inputs["f_b1"], np.float32)
                  + b3n @ np.asarray(inputs["f_w1"], np.float32)).reshape(1, F),
        "w2": b16(inputs["f_w2"]), "b2": b16(inputs["f_b2"]).reshape(1, E),
    }


def get_program(bpc):
    if bpc not in _programs:
        _programs[bpc] = _build(bpc)
    return _programs[bpc]


def kernel(**inputs) -> np.ndarray:
    bpc = B // NCORES
    nc = get_program(bpc)
    in_maps = [_host_prep(inputs, bpc, c) for c in range(NCORES)]
    res = run_bass_kernel_spmd(nc, in_maps, core_ids=list(range(NCORES)))
    out = np.concatenate([res.results[c]["out"] for c in range(NCORES)], axis=0)
    return out.astype(np.float32)



# revision 11
# speedup vs baseline: 1.3176x; 1.0193x over previous
"""Trainium2 Bass kernel for a single transformer decoder layer.

Reference semantics (B=64, T=200, E=512, H=8, D=64):
  x += SelfAttn(LN1(x))   (q,k row-masked by pred_mask, causal)
  x += CrossAttn(LN2(x))  (k from raw memory row-masked by src_mask,
                           v from LN2(x) (!), causal)
  x += FFN(LN3(x))        (512 -> 2048 -> relu -> 512)

Sharding: data-parallel over batch, 8 elems per NeuronCore, no collectives.

Layout strategy (per core, batch elems processed in PAIRS):
  - residual stream x kept NATURAL [t_chunk<=128, 512] in fp32
  - LN via bn_stats/bn_aggr + two fused scalar_tensor_tensor ops
  - activations transposed to [E, 2*T] pair tiles via PE is_transpose
    matmuls (keeps PE warm), DVE drains the PSUM
  - Q,K projected transposed [H*D, 2*T] with weight stationaries, N=400
  - scores computed TRANSPOSED  ST[s, t] = K Q^T  per head per elem,
    2 heads per PSUM bank; exp on ACT (no max subtraction -- scores are
    O(1)); causal mask applied post-exp via gpsimd.affine_select(fill=0)
  - matmul operands must sit at SBUF base partition 0 (row-group-64
    operands crash the device), so odd heads read DMA-shifted copies
  - softmax denominators via one-hot-column matmuls into [8,T] PSUM;
    1/d via reciprocal_approx_fast, broadcast to head halves by a
    one-hot matmul, multiplied into O^T on DVE
  - AV gives O transposed directly (lhsT = V natural slices)
  - biases enter PSUM via rank-1 (K=1) matmuls; FFN b1 rides the
    relu activation bias (per-partition in the transposed layout)
"""

import numpy as np
import ml_dtypes
from contextlib import ExitStack

import concourse.bass as bass
import concourse.bacc as bacc
import concourse.tile as tile
from concourse import mybir
from concourse.bass_utils import run_bass_kernel_spmd

B, T, E, H, Dh, F = 64, 200, 512, 8, 64, 2048
NCORES = 8
SCALE = float(E) ** -0.5
F32 = mybir.dt.float32
BF16 = mybir.dt.bfloat16
AL = mybir.AluOpType
AF = mybir.ActivationFunctionType
TCH = [(0, 128), (128, 72)]  # token chunks (t0, tc)
ECH = E // 128  # 4
FCH = F // 128  # 16
NPBF16 = ml_dtypes.bfloat16

_programs = {}


def _layernorm(nc, pools, x_c, tc, eps):
    """x_c: [tc,512] f32 natural -> (x-mu)*rsqrt(var+eps) as bf16.
    LN gamma is folded into the downstream weights host-side; beta enters
    via rank-1 bias matmuls."""
    st6 = pools["small"].tile([tc, 6], F32, name="st6")
    nc.vector.bn_stats(st6[:, :], x_c)
    mv = pools["small"].tile([tc, 2], F32, name="mv")
    nc.vector.bn_aggr(mv[:, :], st6[:, :])
    std = pools["small"].tile([tc, 1], F32, name="std")
    nc.scalar.activation(std[:, :], mv[:, 1:2], AF.Sqrt, bias=eps[0:tc, 0:1])
    rstd = pools["small"].tile([tc, 1], F32, name="rstd")
    nc.vector.reciprocal(rstd[:, :], std[:, :])
    nb = pools["small"].tile([tc, 1], F32, name="nb")
    nc.vector.tensor_scalar(nb[:, :], mv[:, 0:1], rstd[:, 0:1], -1.0,
                            op0=AL.mult, op1=AL.mult)
    h_c = pools["h"].tile([tc, E], BF16, name="h_c", tag="h_c", bufs=10)
    nc.gpsimd.tensor_scalar(h_c[:, :], x_c, rstd[:, 0:1], nb[:, 0:1],
                            op0=AL.mult, op1=AL.add)
    return h_c


def _transpose_pair(nc, pools, h_cs_pair, ident):
    """h_cs_pair: list of 2 elems x 2 chunks of [tc,512] bf16 natural ->
    hT[ec] [128, 400] bf16 pair tiles via PE transposes."""
    hT = []
    for ec in range(ECH):
        t = pools["tT"].tile([128, 2 * T], BF16, name="hT", bufs=14)
        for el in range(2):
            for ci, (t0, tc) in enumerate(TCH):
                ps = pools["ps"].tile([128, tc], BF16, name="t_ps", tag="ps")
                nc.tensor.transpose(
                    ps[:, :], h_cs_pair[el][ci][0:tc, ec * 128:(ec + 1) * 128],
                    ident[0:tc, 0:tc])
                nc.vector.tensor_copy(t[:, el * T + t0:el * T + t0 + tc], ps[:, :])
        hT.append(t)
    return hT


def _project_qkT(nc, pools, w_sb, rhs_T, name, bcol=None, mask=None):
    """[128, 400] bf16 pair chunks of (W^T h)^T, plus base-partition-0
    copies of rows 64:128 (odd heads must read from partition 0).
    bcol: [128, 4] per-partition bias columns (LN-beta@W, transposed),
    fused into the PSUM drain; mask: [128, 2T] token mask multiplied in."""
    out, hi = [], []
    for oc in range(4):
        ps = pools["ps"].tile([128, 2 * T], F32, name=f"{name}_ps", tag="ps")
        for ec in range(ECH):
            nc.tensor.matmul(ps[:, :], w_sb[:, ec, oc * 128:(oc + 1) * 128],
                             rhs_T[ec][:, :], start=(ec == 0),
                             stop=(ec == 3))
        qk = "q" if name.startswith("q") else "k"
        sb = pools["qkt"].tile([128, 2 * T], BF16, name=f"{name}_sb", tag=qk, bufs=5)
        if bcol is not None and mask is not None:
            nc.vector.scalar_tensor_tensor(sb[:, :], ps[:, :],
                                           bcol[:, oc:oc + 1], mask[:, :],
                                           op0=AL.add, op1=AL.mult)
        elif bcol is not None:
            nc.vector.tensor_scalar(sb[:, :], ps[:, :], bcol[:, oc:oc + 1],
                                    None, op0=AL.add)
        elif mask is not None:
            nc.vector.tensor_mul(sb[:, :], ps[:, :], mask[:, :])
        else:
            nc.vector.tensor_copy(sb[:, :], ps[:, :])
        hb = pools["qkt"].tile([64, 2 * T], BF16, name=f"{name}_hi", tag="hi",
                               bufs=10)
        nc.sync.dma_start(hb[:, :], sb[64:128, :])
        out.append(sb)
        hi.append(hb)
    return out, hi


def _project_v(nc, pools, wv_sb, hT, off, name):
    """v natural [tc, 512] bf16 tiles for ONE elem (lhsT = hT pair slices).
    The LN-beta@Wv bias is folded into the output-projection bias host-side
    (softmax rows sum to 1, so a constant v offset passes straight through)."""
    out = []
    for (t0, tc) in TCH:
        ps = pools["ps"].tile([tc, E], F32, name=f"{name}_ps", tag="ps")
        for ec in range(ECH):
            nc.tensor.matmul(ps[:, :], hT[ec][:, off + t0:off + t0 + tc],
                             wv_sb[:, ec, :], start=(ec == 0),
                             stop=(ec == 3))
        sb = pools["v"].tile([tc, E], BF16, name=f"{name}_sb", tag="v", bufs=6)
        nc.scalar.copy(sb[:, :], ps[:, :])
        out.append(sb)
    return out


def _attention(nc, pools, qkt, v_sb, ones64, wo_sb, bo_row, ones_row,
               x_cs, off):
    """Causal attention for ONE elem (token cols off:off+200 of the pair
    tiles) + output projection + bias + residual."""
    (qT_lo, qT_hi), (kT_lo, kT_hi) = qkt
    e0m, e1m = [], []
    # pass A: scores (transposed), exp, causal select; 2 heads per psum bank
    for oc in range(4):
        st0 = pools["ps"].tile([128, 2, 200], F32, name="st0", tag="ps")
        st1 = pools["ps"].tile([72, 2, 72], F32, name="st1", tag="ps")
        for hl in range(2):
            qh = (qT_lo, qT_hi)[hl][oc][0:64, off:off + 200]
            kh = (kT_lo, kT_hi)[hl][oc][0:64, off:off + 200]
            nc.tensor.matmul(st0[:, hl, :], kh[:, 0:128], qh)
            nc.tensor.matmul(st1[:, hl, :], kh[:, 128:200], qh[:, 128:200])
        e0 = pools["e0"].tile([128, 2, 200], BF16, name="e0", bufs=3)
        nc.scalar.activation(e0[:, :, :], st0[:, :, :], AF.Exp, scale=SCALE)
        e1 = pools["e1"].tile([72, 2, 72], BF16, name="e1", bufs=3)
        nc.scalar.activation(e1[:, :, :], st1[:, :, :], AF.Exp, scale=SCALE)
        # causal: keep where t - s >= 0 (iota = -p + t), else 0
        e0x = pools["e0"].tile([128, 2, 200], BF16, name="e0x", bufs=5)
        nc.gpsimd.affine_select(
            e0x[:, :, :], e0[:, :, :], pattern=[[0, 2], [1, 200]],
            compare_op=AL.is_ge, fill=0.0, base=0, channel_multiplier=-1)
        e1x = pools["e1"].tile([72, 2, 72], BF16, name="e1x", bufs=5)
        nc.gpsimd.affine_select(
            e1x[:, :, :], e1[:, :, :], pattern=[[0, 2], [1, 72]],
            compare_op=AL.is_ge, fill=0.0, base=0, channel_multiplier=-1)
        e0m.append(e0x)
        e1m.append(e1x)
    # pass B+C: per oc, broadcast-summed denominators via an all-ones
    # stationary (every output row = sum_s e[s,t], landing on the head's
    # partition range directly), reciprocal in-place, then O^T = V^T @ E^T
    oT_sb = []
    for oc in range(4):
        dbc_ps = pools["ps"].tile([128, 200], F32, name="dbc_ps", tag="ps")
        nc.tensor.matmul(dbc_ps[0:64, 0:200], ones64[0:128, :],
                         e0m[oc][:, 0, :], start=True, stop=False,
                         skip_group_check=True)
        nc.tensor.matmul(dbc_ps[64:128, 0:200], ones64[0:128, :],
                         e0m[oc][:, 1, :], start=True, stop=False,
                         skip_group_check=True)
        nc.tensor.matmul(dbc_ps[0:64, 128:200], ones64[0:72, :],
                         e1m[oc][:, 0, :], start=False, stop=False,
                         skip_group_check=True)
        nc.tensor.matmul(dbc_ps[64:128, 128:200], ones64[0:72, :],
                         e1m[oc][:, 1, :], start=False, stop=True,
                         skip_group_check=True)
        dbc = pools["dbc"].tile([128, 200], F32, name="dbc")
        nc.vector.reciprocal_approx_fast(dbc[:, :], dbc_ps[:, :])
        ot_ps = pools["ps"].tile([128, 200], F32, name="ot_ps", tag="ps")
        for hl in range(2):
            h = 2 * oc + hl
            hp = hl * 64
            nc.tensor.matmul(ot_ps[hp:hp + 64, 0:200],
                             v_sb[0][0:128, h * 64:(h + 1) * 64],
                             e0m[oc][:, hl, :], start=True, stop=False,
                             skip_group_check=True)
            nc.tensor.matmul(ot_ps[hp:hp + 64, 128:200],
                             v_sb[1][0:72, h * 64:(h + 1) * 64],
                             e1m[oc][:, hl, :], start=False, stop=True,
                             skip_group_check=True)
        ot = pools["ot"].tile([128, 200], BF16, name="ot", bufs=6)
        nc.vector.tensor_mul(ot[:, :], ot_ps[:, :], dbc[:, :])
        oT_sb.append(ot)
    # output projection (natural) + bias via rank-1 matmul + residual
    new_x = []
    for ci, (t0, tc) in enumerate(TCH):
        ps = pools["ps"].tile([tc, E], F32, name="proj_ps", tag="ps")
        for hc in range(4):
            nc.tensor.matmul(ps[:, :], oT_sb[hc][:, t0:t0 + tc],
                             wo_sb[:, hc, :], start=(hc == 0), stop=False)
        nc.tensor.matmul(ps[:, :], ones_row[0:1, 0:tc], bo_row[0:1, :],
                         start=False, stop=True)
        xn = pools["res"].tile([tc, E], F32, name="xn", tag="res")
        nc.vector.tensor_add(xn[:, :], ps[:, :], x_cs[ci])
        new_x.append(xn)
    return new_x


def _build(bpc, stages=3):
    nc = bacc.Bacc("TRN2", target_bir_lowering=False, debug=False,
                   enable_asserts=False, num_devices=NCORES)
    dram = {}

    def din(name, shape, dt):
        h = nc.dram_tensor(name, list(shape), dt, kind="ExternalInput")
        dram[name] = h
        return h

    x_d = din("x", (bpc, T, E), F32)
    mem_d = din("mem", (bpc, T, E), BF16)
    pm_d = din("pm", (bpc, T), BF16)
    sm_d = din("sm", (bpc, T), BF16)
    wq_sa_d = din("wq_sa", (E, E), BF16)
    wk_sa_d = din("wk_sa", (E, E), BF16)
    wv_sa_d = din("wv_sa", (E, E), BF16)
    wo_sa_d = din("wo_sa", (E, E), BF16)
    bo_sa_d = din("bo_sa", (1, E), BF16)
    wq_ca_d = din("wq_ca", (E, E), BF16)
    wk_ca_d = din("wk_ca", (E, E), BF16)
    wv_ca_d = din("wv_ca", (E, E), BF16)
    wo_ca_d = din("wo_ca", (E, E), BF16)
    bo_ca_d = din("bo_ca", (1, E), BF16)
    w1_d = din("w1", (E, F), BF16)
    b1_d = din("b1", (1, F), BF16)
    w2_d = din("w2", (F, E), BF16)
    b2_d = din("b2", (1, E), BF16)
    bq_sa_d = din("bq_sa", (128, 4), F32)
    bk_sa_d = din("bk_sa", (128, 4), F32)
    bq_ca_d = din("bq_ca", (128, 4), F32)
    out_d = nc.dram_tensor("out", [bpc, T, E], F32, kind="ExternalOutput")

    ones_d = nc.inline_tensor(np.ones((1, E), dtype=NPBF16), name="onesc")
    ones64_d = nc.inline_tensor(np.ones((128, 64), dtype=NPBF16), name="ones64c")
    identb_d = nc.inline_tensor(np.eye(128, dtype=NPBF16), name="identbc")

    with tile.TileContext(nc) as tcx, ExitStack() as ctx:
        pools = {}

        def pool(name, bufs, space="SBUF"):
            pools[name] = ctx.enter_context(
                tcx.tile_pool(name=name, bufs=bufs, space=space))
            return pools[name]

        wpool = pool("w", 1)
        pool("small", 12)
        pool("h", 10)
        pool("tT", 14)
        pool("qkt", 6)
        pool("v", 6)
        pool("e0", 3)
        pool("e1", 3)
        pool("ot", 6)
        pool("dbc", 4)
        pool("res", 20)
        pool("rT", 17)
        pool("mrow", 4)
        pool("mbc", 5)
        pool("ps", 8, space="PSUM")

        def wtile(name, src, shape, rearr=None, dt=BF16, eng=None):
            t = wpool.tile(shape, dt, tag=name, bufs=1, name=name)
            ap = src[:] if rearr is None else src[:].rearrange(rearr, p=128)
            (eng or nc.sync).dma_start(t[...], ap)
            return t

        # SA weights first (sync queue) so pair 0 starts quickly; bulk
        # FFN/CA weights go on the scalar HWDGE queue in parallel
        identb = wtile("identb", identb_d, [128, 128])
        ones64 = wtile("ones64", ones64_d, [128, 64])
        ones_row = wtile("ones", ones_d, [1, E])
        wq_sa = wtile("wq_sa", wq_sa_d, [128, ECH, E], "(c p) n -> p c n")
        wk_sa = wtile("wk_sa", wk_sa_d, [128, ECH, E], "(c p) n -> p c n")
        wv_sa = wtile("wv_sa", wv_sa_d, [128, ECH, E], "(c p) n -> p c n")
        wo_sa = wtile("wo_sa", wo_sa_d, [128, ECH, E], "(c p) n -> p c n")
        bq_sa = wtile("bq_sa", bq_sa_d, [128, 4], dt=F32)
        bk_sa = wtile("bk_sa", bk_sa_d, [128, 4], dt=F32)
        bo_sa = wtile("bo_sa", bo_sa_d, [1, E])
        wq_ca = wtile("wq_ca", wq_ca_d, [128, ECH, E], "(c p) n -> p c n",
                      eng=nc.scalar)
        wk_ca = wtile("wk_ca", wk_ca_d, [128, ECH, E], "(c p) n -> p c n",
                      eng=nc.scalar)
        wv_ca = wtile("wv_ca", wv_ca_d, [128, ECH, E], "(c p) n -> p c n",
                      eng=nc.scalar)
        wo_ca = wtile("wo_ca", wo_ca_d, [128, ECH, E], "(c p) n -> p c n",
                      eng=nc.scalar)
        bq_ca = wtile("bq_ca", bq_ca_d, [128, 4], dt=F32, eng=nc.scalar)
        bo_ca = wtile("bo_ca", bo_ca_d, [1, E], eng=nc.scalar)
        w1 = wtile("w1", w1_d, [128, ECH, F], "(c p) n -> p c n", eng=nc.scalar)
        w2 = wtile("w2", w2_d, [128, FCH, E], "(c p) n -> p c n", eng=nc.scalar)
        b2r = wtile("b2", b2_d, [1, E], eng=nc.scalar)
        # f_b1 (+ folded ln3_b @ w1) in column layout for the relu bias
        b1c = wpool.tile([128, FCH], F32, tag="b1c", bufs=1, name="b1c")
        b1cb = wpool.tile([128, FCH], BF16, tag="b1cb", bufs=1, name="b1cb")
        nc.scalar.dma_start(b1cb[...],
                            b1_d[:].rearrange("o (c p) -> p (o c)", p=128))
        nc.vector.tensor_copy(b1c[:, :], b1cb[:, :])
        eps = wpool.tile([128, 1], F32, tag="eps", bufs=1, name="eps")
        nc.gpsimd.memset(eps[:, :], 1e-5)

        for pr in range(bpc // 2):
            els = (2 * pr, 2 * pr + 1)
            # ---- load x and masks for both elems ----
            x_el = []
            pm2 = pools["mbc"].tile([128, 2 * T], BF16, name="pm2")
            sm2 = pools["mbc"].tile([128, 2 * T], BF16, name="sm2")
            for el, e in enumerate(els):
                x_cs = []
                for (t0, tc) in TCH:
                    xt = pools["res"].tile([tc, E], F32, name="x_in", tag="res", bufs=20)
                    nc.sync.dma_start(xt[:, :], x_d[e, t0:t0 + tc, :])
                    x_cs.append(xt)
                x_el.append(x_cs)
                pm_row = pools["mrow"].tile([1, T], BF16, name="pm_row", bufs=2)
                nc.sync.dma_start(pm_row[:, :], pm_d[e:e + 1, :])
                nc.gpsimd.partition_broadcast(pm2[:, el * T:(el + 1) * T],
                                              pm_row[:, :])
                sm_row = pools["mrow"].tile([1, T], BF16, name="sm_row", bufs=2)
                nc.sync.dma_start(sm_row[:, :], sm_d[e:e + 1, :])
                nc.gpsimd.partition_broadcast(sm2[:, el * T:(el + 1) * T],
                                              sm_row[:, :])

            # ======== self-attention ========
            h_pair = [[_layernorm(nc, pools, x_el[el][ci][:, :], tc, eps)
                       for ci, (t0, tc) in enumerate(TCH)] for el in range(2)]
            hT = _transpose_pair(nc, pools, h_pair, identb)
            qT = _project_qkT(nc, pools, wq_sa, hT, "q_sa", bq_sa, pm2)
            kT = _project_qkT(nc, pools, wk_sa, hT, "k_sa", bk_sa, pm2)
            for el in range(2):
                v_sb = _project_v(nc, pools, wv_sa, hT, el * T, "v_sa")
                x_el[el] = _attention(nc, pools, (qT, kT), v_sb, ones64,
                                      wo_sa, bo_sa, ones_row, x_el[el], el * T)
            if stages == 1:
                for el, e in enumerate(els):
                    for ci, (t0, tc) in enumerate(TCH):
                        nc.sync.dma_start(out_d[e, t0:t0 + tc, :],
                                          x_el[el][ci][:, :])
                continue

            # ======== cross-attention ========
            h_pair = [[_layernorm(nc, pools, x_el[el][ci][:, :], tc, eps)
                       for ci, (t0, tc) in enumerate(TCH)] for el in range(2)]
            h2T = _transpose_pair(nc, pools, h_pair, identb)
            m_pair = []
            for el, e in enumerate(els):
                m_cs = []
                for (t0, tc) in TCH:
                    mt = pools["h"].tile([tc, E], BF16, name="m_nat",
                                         tag="m_nat", bufs=6)
                    nc.sync.dma_start(mt[:, :], mem_d[e, t0:t0 + tc, :])
                    m_cs.append(mt)
                m_pair.append(m_cs)
            mT = _transpose_pair(nc, pools, m_pair, identb)
            qT = _project_qkT(nc, pools, wq_ca, h2T, "q_ca", bq_ca)
            kT = _project_qkT(nc, pools, wk_ca, mT, "k_ca", mask=sm2)
            for el in range(2):
                v_sb = _project_v(nc, pools, wv_ca, h2T, el * T, "v_ca")
                x_el[el] = _attention(nc, pools, (qT, kT), v_sb, ones64,
                                      wo_ca, bo_ca, ones_row, x_el[el], el * T)
            if stages == 2:
                for el, e in enumerate(els):
                    for ci, (t0, tc) in enumerate(TCH):
                        nc.sync.dma_start(out_d[e, t0:t0 + tc, :],
                                          x_el[el][ci][:, :])
                continue

            # ======== feed-forward ========
            h_pair = [[_layernorm(nc, pools, x_el[el][ci][:, :], tc, eps)
                       for ci, (t0, tc) in enumerate(TCH)] for el in range(2)]
            h3T = _transpose_pair(nc, pools, h_pair, identb)
            rT = []
            for fc in range(FCH):
                zps = pools["ps"].tile([128, 2 * T], F32, name="z_ps",
                                          tag="ps")
                for ec in range(ECH):
                    nc.tensor.matmul(zps[:, :],
                                     w1[:, ec, fc * 128:(fc + 1) * 128],
                                     h3T[ec][:, :], start=(ec == 0),
                                     stop=(ec == 3))
                r = pools["rT"].tile([128, 2 * T], BF16, name="r")
                nc.scalar.activation(r[:, :], zps[:, :], AF.Relu,
                                     bias=b1c[:, fc:fc + 1])
                rT.append(r)
            for el, e in enumerate(els):
                for ci, (t0, tc) in enumerate(TCH):
                    yps = pools["ps"].tile([tc, E], F32, name="y_ps",
                                                tag="ps")
                    for fc in range(FCH):
                        nc.tensor.matmul(yps[:, :],
                                         rT[fc][:, el * T + t0:el * T + t0 + tc],
                                         w2[:, fc, :], start=(fc == 0),
                                         stop=False)
                    nc.tensor.matmul(yps[:, :], ones_row[0:1, 0:tc],
                                     b2r[0:1, :], start=False, stop=True)
                    yout = pools["res"].tile([tc, E], F32, name="yout",
                                             tag="res")
                    nc.vector.tensor_add(yout[:, :], yps[:, :],
                                         x_el[el][ci][:, :])
                    nc.sync.dma_start(out_d[e, t0:t0 + tc, :], yout[:, :])

    nc.compile()
    return nc


def _host_prep(inputs, bpc, core):
    """Build the in_map for one core."""
    s = slice(core * bpc, (core + 1) * bpc)

    def rearr(w, g=None):  # (H, E, D) -> [E, H*D], optionally row-scaled
        m = np.transpose(np.asarray(w, np.float32), (1, 0, 2)).reshape(E, E)
        if g is not None:
            m = m * np.asarray(g, np.float32)[:, None]
        return np.ascontiguousarray(m).astype(NPBF16)

    def b16(a):
        return np.ascontiguousarray(np.asarray(a, np.float32)).astype(NPBF16)

    def f32c(a):
        return np.ascontiguousarray(np.asarray(a, np.float32))

    g1 = np.asarray(inputs["ln1_g"], np.float32)
    b1n = np.asarray(inputs["ln1_b"], np.float32)
    g2 = np.asarray(inputs["ln2_g"], np.float32)
    b2n = np.asarray(inputs["ln2_b"], np.float32)
    g3 = np.asarray(inputs["ln3_g"], np.float32)
    b3n = np.asarray(inputs["ln3_b"], np.float32)

    def wr(w):  # raw rearranged fp32 (for beta @ W rows)
        return np.transpose(np.asarray(w, np.float32), (1, 0, 2)).reshape(E, E)

    def bcolT(b):  # [E] bias row -> [128, 4] per-partition columns per oc
        return np.ascontiguousarray(
            np.asarray(b, np.float32).reshape(4, 128).T)

    wo_sa_f = np.asarray(inputs["sa_wo"], np.float32)
    wo_ca_f = np.asarray(inputs["ca_wo"], np.float32)
    bv_sa = b1n @ wr(inputs["sa_wv"])
    bv_ca = b2n @ wr(inputs["ca_wv"])

    return {
        "x": f32c(inputs["idx"][s]),
        "mem": b16(inputs["memory"][s]),
        "pm": b16(inputs["pred_mask"][s] != 0),
        "sm": b16(inputs["src_mask"][s] != 0),
        "wq_sa": rearr(inputs["sa_wq"], g1), "wk_sa": rearr(inputs["sa_wk"], g1),
        "wv_sa": rearr(inputs["sa_wv"], g1),
        "wo_sa": b16(inputs["sa_wo"]),
        "bo_sa": b16(np.asarray(inputs["sa_bo"], np.float32)
                     + bv_sa @ wo_sa_f).reshape(1, E),
        "bq_sa": bcolT(b1n @ wr(inputs["sa_wq"])),
        "bk_sa": bcolT(b1n @ wr(inputs["sa_wk"])),
        "wq_ca": rearr(inputs["ca_wq"], g2), "wk_ca": rearr(inputs["ca_wk"]),
        "wv_ca": rearr(inputs["ca_wv"], g2),
        "wo_ca": b16(inputs["ca_wo"]),
        "bo_ca": b16(np.asarray(inputs["ca_bo"], np.float32)
                     + bv_ca @ wo_ca_f).reshape(1, E),
        "bq_ca": bcolT(b2n @ wr(inputs["ca_wq"])),
        "w1": b16(np.asarray(inputs["f_w1"], np.float32)
                  * g3[:, None]),
        "b1": b16(np.asarray(inputs["f_b1"], np.float32)
                  + b3n @ np.asarray(inputs["f_w1"], np.float32)).reshape(1, F),
        "w2": b16(inputs["f_w2"]), "b2": b16(inputs["f_b2"]).reshape(1, E),
    }


def get_program(bpc):
    if bpc not in _programs:
        _programs[bpc] = _build(bpc)
    return _programs[bpc]


def kernel(**inputs) -> np.ndarray:
    bpc = B // NCORES
    nc = get_program(bpc)
    in_maps = [_host_prep(inputs, bpc, c) for c in range(NCORES)]
    res = run_bass_kernel_spmd(nc, in_maps, core_ids=list(range(NCORES)))
    out = np.concatenate([res.results[c]["out"] for c in range(NCORES)], axis=0)
    return out.astype(np.float32)



# revision 16
# speedup vs baseline: 1.4742x; 1.1189x over previous
"""Trainium2 Bass kernel for a single transformer decoder layer.

Reference semantics (B=64, T=200, E=512, H=8, D=64):
  x += SelfAttn(LN1(x))   (q,k row-masked by pred_mask, causal)
  x += CrossAttn(LN2(x))  (k from raw memory row-masked by src_mask,
                           v from LN2(x) (!), causal)
  x += FFN(LN3(x))        (512 -> 2048 -> relu -> 512)

Sharding: data-parallel over batch, 8 elems per NeuronCore, no collectives.

Layout strategy (per core, batch elems processed in PAIRS):
  - residual stream x kept NATURAL [t_chunk<=128, 512] in fp32
  - LN via bn_stats/bn_aggr + two fused scalar_tensor_tensor ops
  - activations transposed to [E, 2*T] pair tiles via PE is_transpose
    matmuls (keeps PE warm), DVE drains the PSUM
  - Q,K projected transposed [H*D, 2*T] with weight stationaries, N=400
  - scores computed TRANSPOSED  ST[s, t] = K Q^T  per head per elem,
    2 heads per PSUM bank; exp on ACT (no max subtraction -- scores are
    O(1)); causal mask applied post-exp via gpsimd.affine_select(fill=0)
  - matmul operands must sit at SBUF base partition 0 (row-group-64
    operands crash the device), so odd heads read DMA-shifted copies
  - softmax denominators via one-hot-column matmuls into [8,T] PSUM;
    1/d via reciprocal_approx_fast, broadcast to head halves by a
    one-hot matmul, multiplied into O^T on DVE
  - AV gives O transposed directly (lhsT = V natural slices)
  - biases enter PSUM via rank-1 (K=1) matmuls; FFN b1 rides the
    relu activation bias (per-partition in the transposed layout)
"""

import numpy as np
import ml_dtypes
from contextlib import ExitStack

import concourse.bass as bass
import concourse.bacc as bacc
import concourse.tile as tile
from concourse import mybir
from concourse.bass_utils import run_bass_kernel_spmd

B, T, E, H, Dh, F = 64, 200, 512, 8, 64, 2048
NCORES = 8
SCALE = float(E) ** -0.5
F32 = mybir.dt.float32
BF16 = mybir.dt.bfloat16
AL = mybir.AluOpType
AF = mybir.ActivationFunctionType
TCH = [(0, 128), (128, 72)]  # token chunks (t0, tc)
ECH = E // 128  # 4
FCH = F // 128  # 16
NPBF16 = ml_dtypes.bfloat16

_programs = {}


def _layernorm(nc, pools, x_c, tc, eps):
    """x_c: [tc,512] f32 natural -> (x-mu)*rsqrt(var+eps) as bf16.
    LN gamma is folded into the downstream weights host-side; beta enters
    via rank-1 bias matmuls."""
    st6 = pools["small"].tile([tc, 6], F32, name="st6")
    nc.vector.bn_stats(st6[:, :], x_c)
    mv = pools["small"].tile([tc, 2], F32, name="mv")
    nc.vector.bn_aggr(mv[:, :], st6[:, :])
    std = pools["small"].tile([tc, 1], F32, name="std")
    nc.scalar.activation(std[:, :], mv[:, 1:2], AF.Sqrt, bias=eps[0:tc, 0:1])
    rstd = pools["small"].tile([tc, 1], F32, name="rstd")
    nc.vector.reciprocal(rstd[:, :], std[:, :])
    nb = pools["small"].tile([tc, 1], F32, name="nb")
    nc.vector.tensor_scalar(nb[:, :], mv[:, 0:1], rstd[:, 0:1], -1.0,
                            op0=AL.mult, op1=AL.mult)
    h_c = pools["h"].tile([tc, E], BF16, name="h_c", tag="h_c", bufs=10)
    nc.gpsimd.tensor_scalar(h_c[:, :], x_c, rstd[:, 0:1], nb[:, 0:1],
                            op0=AL.mult, op1=AL.add)
    return h_c


def _transpose_pair(nc, pools, h_cs_pair, ident):
    """h_cs_pair: list of 2 elems x 2 chunks of [tc,512] bf16 natural ->
    hT[ec] [128, 400] bf16 pair tiles via PE transposes."""
    hT = []
    for ec in range(ECH):
        t = pools["tT"].tile([128, 2 * T], BF16, name="hT", bufs=15)
        for el in range(2):
            for ci, (t0, tc) in enumerate(TCH):
                ps = pools["ps"].tile([128, tc], BF16, name="t_ps", tag="ps")
                nc.tensor.transpose(
                    ps[:, :], h_cs_pair[el][ci][0:tc, ec * 128:(ec + 1) * 128],
                    ident[0:tc, 0:tc])
                nc.vector.tensor_copy(t[:, el * T + t0:el * T + t0 + tc], ps[:, :])
        hT.append(t)
    return hT


def _project_qkT(nc, pools, w_sb, rhs_T, name, bcol=None, mask=None):
    """[128, 400] bf16 pair chunks of (W^T h)^T, plus base-partition-0
    copies of rows 64:128 (odd heads must read from partition 0).
    bcol: [128, 4] per-partition bias columns (LN-beta@W, transposed),
    fused into the PSUM drain; mask: [128, 2T] token mask multiplied in."""
    out, hi = [], []
    for oc in range(4):
        ps = pools["ps"].tile([128, 2 * T], F32, name=f"{name}_ps", tag="ps")
        for ec in range(ECH):
            nc.tensor.matmul(ps[:, :], w_sb[:, ec, oc * 128:(oc + 1) * 128],
                             rhs_T[ec][:, :], start=(ec == 0),
                             stop=(ec == 3))
        qk = "q" if name.startswith("q") else "k"
        sb = pools["qkt"].tile([128, 2 * T], BF16, name=f"{name}_sb", tag=qk, bufs=8)
        if bcol is not None and mask is not None:
            nc.vector.scalar_tensor_tensor(sb[:, :], ps[:, :],
                                           bcol[:, oc:oc + 1], mask[:, :],
                                           op0=AL.add, op1=AL.mult)
        elif bcol is not None:
            nc.vector.tensor_scalar(sb[:, :], ps[:, :], bcol[:, oc:oc + 1],
                                    None, op0=AL.add)
        elif mask is not None:
            nc.vector.tensor_mul(sb[:, :], ps[:, :], mask[:, :])
        else:
            nc.vector.tensor_copy(sb[:, :], ps[:, :])
        hb = pools["qkt"].tile([64, 2 * T], BF16, name=f"{name}_hi", tag="hi",
                               bufs=10)
        nc.sync.dma_start(hb[:, :], sb[64:128, :])
        out.append(sb)
        hi.append(hb)
    return out, hi


def _project_v(nc, pools, wv_sb, hT, off, name):
    """v natural [tc, 512] bf16 tiles for ONE elem (lhsT = hT pair slices).
    The LN-beta@Wv bias is folded into the output-projection bias host-side
    (softmax rows sum to 1, so a constant v offset passes straight through)."""
    out = []
    for (t0, tc) in TCH:
        ps = pools["ps"].tile([tc, E], F32, name=f"{name}_ps", tag="ps")
        for ec in range(ECH):
            nc.tensor.matmul(ps[:, :], hT[ec][:, off + t0:off + t0 + tc],
                             wv_sb[:, ec, :], start=(ec == 0),
                             stop=(ec == 3))
        sb = pools["v"].tile([tc, E], BF16, name=f"{name}_sb", tag="v", bufs=5)
        nc.scalar.copy(sb[:, :], ps[:, :])
        out.append(sb)
    return out


def _attention(nc, pools, qkt, v_sb, ones64, wo_sb, bo_row, ones_row,
               x_cs, off):
    """Causal attention for ONE elem (token cols off:off+200 of the pair
    tiles) + output projection + bias + residual."""
    (qT_lo, qT_hi), (kT_lo, kT_hi) = qkt
    e0m, e1m = [], []
    # pass A: scores (transposed), exp, causal select; 2 heads per psum bank
    for oc in range(4):
        st0 = pools["ps"].tile([128, 2, 200], F32, name="st0", tag="ps")
        st1 = pools["ps"].tile([72, 2, 72], F32, name="st1", tag="ps")
        for hl in range(2):
            qh = (qT_lo, qT_hi)[hl][oc][0:64, off:off + 200]
            kh = (kT_lo, kT_hi)[hl][oc][0:64, off:off + 200]
            nc.tensor.matmul(st0[:, hl, :], kh[:, 0:128], qh)
            nc.tensor.matmul(st1[:, hl, :], kh[:, 128:200], qh[:, 128:200])
        e0 = pools["e0"].tile([128, 2, 200], BF16, name="e0", bufs=4)
        nc.scalar.activation(e0[:, :, :], st0[:, :, :], AF.Exp, scale=SCALE)
        e1 = pools["e1"].tile([72, 2, 72], BF16, name="e1", bufs=5)
        nc.scalar.activation(e1[:, :, :], st1[:, :, :], AF.Exp, scale=SCALE)
        # causal: keep where t - s >= 0 (iota = -p + t), else 0
        e0x = pools["e0"].tile([128, 2, 200], BF16, name="e0x", bufs=7)
        nc.gpsimd.affine_select(
            e0x[:, :, :], e0[:, :, :], pattern=[[0, 2], [1, 200]],
            compare_op=AL.is_ge, fill=0.0, base=0, channel_multiplier=-1)
        e1x = pools["e1"].tile([72, 2, 72], BF16, name="e1x", bufs=8)
        nc.gpsimd.affine_select(
            e1x[:, :, :], e1[:, :, :], pattern=[[0, 2], [1, 72]],
            compare_op=AL.is_ge, fill=0.0, base=0, channel_multiplier=-1)
        e0m.append(e0x)
        e1m.append(e1x)
    # pass B+C: per oc, broadcast-summed denominators via an all-ones
    # stationary (every output row = sum_s e[s,t], landing on the head's
    # partition range directly), reciprocal in-place, then O^T = V^T @ E^T
    oT_sb = []
    for oc in range(4):
        dbc_ps = pools["ps"].tile([128, 200], F32, name="dbc_ps", tag="ps")
        nc.tensor.matmul(dbc_ps[0:64, 0:200], ones64[0:128, :],
                         e0m[oc][:, 0, :], start=True, stop=False,
                         skip_group_check=True)
        nc.tensor.matmul(dbc_ps[64:128, 0:200], ones64[0:128, :],
                         e0m[oc][:, 1, :], start=True, stop=False,
                         skip_group_check=True)
        nc.tensor.matmul(dbc_ps[0:64, 128:200], ones64[0:72, :],
                         e1m[oc][:, 0, :], start=False, stop=False,
                         skip_group_check=True)
        nc.tensor.matmul(dbc_ps[64:128, 128:200], ones64[0:72, :],
                         e1m[oc][:, 1, :], start=False, stop=True,
                         skip_group_check=True)
        dbc = pools["dbc"].tile([128, 200], F32, name="dbc")
        nc.vector.reciprocal_approx_fast(dbc[:, :], dbc_ps[:, :])
        ot_ps = pools["ps"].tile([128, 200], F32, name="ot_ps", tag="ps")
        for hl in range(2):
            h = 2 * oc + hl
            hp = hl * 64
            nc.tensor.matmul(ot_ps[hp:hp + 64, 0:200],
                             v_sb[0][0:128, h * 64:(h + 1) * 64],
                             e0m[oc][:, hl, :], start=True, stop=False,
                             skip_group_check=True)
            nc.tensor.matmul(ot_ps[hp:hp + 64, 128:200],
                             v_sb[1][0:72, h * 64:(h + 1) * 64],
                             e1m[oc][:, hl, :], start=False, stop=True,
                             skip_group_check=True)
        ot = pools["ot"].tile([128, 200], BF16, name="ot", bufs=7)
        nc.vector.tensor_mul(ot[:, :], ot_ps[:, :], dbc[:, :])
        oT_sb.append(ot)
    # output projection (natural) + bias via rank-1 matmul + residual
    new_x = []
    for ci, (t0, tc) in enumerate(TCH):
        ps = pools["ps"].tile([tc, E], F32, name="proj_ps", tag="ps")
        for hc in range(4):
            nc.tensor.matmul(ps[:, :], oT_sb[hc][:, t0:t0 + tc],
                             wo_sb[:, hc, :], start=(hc == 0), stop=False)
        nc.tensor.matmul(ps[:, :], ones_row[0:1, 0:tc], bo_row[0:1, :],
                         start=False, stop=True)
        xn = pools["res"].tile([tc, E], F32, name="xn", tag="res")
        nc.vector.tensor_add(xn[:, :], ps[:, :], x_cs[ci])
        new_x.append(xn)
    return new_x


def _build(bpc, stages=3):
    nc = bacc.Bacc("TRN2", target_bir_lowering=False, debug=False,
                   enable_asserts=False, num_devices=NCORES)
    dram = {}

    def din(name, shape, dt):
        h = nc.dram_tensor(name, list(shape), dt, kind="ExternalInput")
        dram[name] = h
        return h

    x_d = din("x", (bpc, T, E), F32)
    mem_d = din("mem", (bpc, T, E), BF16)
    pm_d = din("pm", (bpc, T), BF16)
    sm_d = din("sm", (bpc, T), BF16)
    wq_sa_d = din("wq_sa", (E, E), BF16)
    wk_sa_d = din("wk_sa", (E, E), BF16)
    wv_sa_d = din("wv_sa", (E, E), BF16)
    wo_sa_d = din("wo_sa", (E, E), BF16)
    bo_sa_d = din("bo_sa", (1, E), BF16)
    wq_ca_d = din("wq_ca", (E, E), BF16)
    wk_ca_d = din("wk_ca", (E, E), BF16)
    wv_ca_d = din("wv_ca", (E, E), BF16)
    wo_ca_d = din("wo_ca", (E, E), BF16)
    bo_ca_d = din("bo_ca", (1, E), BF16)
    w1_d = din("w1", (E, F), BF16)
    b1_d = din("b1", (1, F), BF16)
    w2_d = din("w2", (F, E), BF16)
    b2_d = din("b2", (1, E), BF16)
    bq_sa_d = din("bq_sa", (128, 4), F32)
    bk_sa_d = din("bk_sa", (128, 4), F32)
    bq_ca_d = din("bq_ca", (128, 4), F32)
    out_d = nc.dram_tensor("out", [bpc, T, E], F32, kind="ExternalOutput")

    ones_d = nc.inline_tensor(np.ones((1, E), dtype=NPBF16), name="onesc")
    ones64_d = nc.inline_tensor(np.ones((128, 64), dtype=NPBF16), name="ones64c")
    identb_d = nc.inline_tensor(np.eye(128, dtype=NPBF16), name="identbc")

    with tile.TileContext(nc) as tcx, ExitStack() as ctx:
        pools = {}

        def pool(name, bufs, space="SBUF"):
            pools[name] = ctx.enter_context(
                tcx.tile_pool(name=name, bufs=bufs, space=space))
            return pools[name]

        wpool = pool("w", 1)
        pool("small", 12)
        pool("h", 10)
        pool("tT", 14)
        pool("qkt", 6)
        pool("v", 6)
        pool("e0", 3)
        pool("e1", 3)
        pool("ot", 6)
        pool("dbc", 4)
        pool("res", 20)
        pool("rT", 16)
        pool("mrow", 4)
        pool("mbc", 6)
        pool("ps", 8, space="PSUM")

        def wtile(name, src, shape, rearr=None, dt=BF16, eng=None):
            t = wpool.tile(shape, dt, tag=name, bufs=1, name=name)
            ap = src[:] if rearr is None else src[:].rearrange(rearr, p=128)
            (eng or nc.sync).dma_start(t[...], ap)
            return t

        # SA weights first (sync queue) so pair 0 starts quickly; bulk
        # FFN/CA weights go on the scalar HWDGE queue in parallel
        identb = wtile("identb", identb_d, [128, 128])
        ones64 = wtile("ones64", ones64_d, [128, 64])
        ones_row = wtile("ones", ones_d, [1, E])
        wq_sa = wtile("wq_sa", wq_sa_d, [128, ECH, E], "(c p) n -> p c n")
        wk_sa = wtile("wk_sa", wk_sa_d, [128, ECH, E], "(c p) n -> p c n")
        wv_sa = wtile("wv_sa", wv_sa_d, [128, ECH, E], "(c p) n -> p c n")
        wo_sa = wtile("wo_sa", wo_sa_d, [128, ECH, E], "(c p) n -> p c n")
        bq_sa = wtile("bq_sa", bq_sa_d, [128, 4], dt=F32)
        bk_sa = wtile("bk_sa", bk_sa_d, [128, 4], dt=F32)
        bo_sa = wtile("bo_sa", bo_sa_d, [1, E])
        wq_ca = wtile("wq_ca", wq_ca_d, [128, ECH, E], "(c p) n -> p c n",
                      eng=nc.scalar)
        wk_ca = wtile("wk_ca", wk_ca_d, [128, ECH, E], "(c p) n -> p c n",
                      eng=nc.scalar)
        wv_ca = wtile("wv_ca", wv_ca_d, [128, ECH, E], "(c p) n -> p c n",
                      eng=nc.scalar)
        wo_ca = wtile("wo_ca", wo_ca_d, [128, ECH, E], "(c p) n -> p c n",
                      eng=nc.scalar)
        bq_ca = wtile("bq_ca", bq_ca_d, [128, 4], dt=F32, eng=nc.scalar)
        bo_ca = wtile("bo_ca", bo_ca_d, [1, E], eng=nc.scalar)
        w1 = wtile("w1", w1_d, [128, ECH, F], "(c p) n -> p c n", eng=nc.scalar)
        w2 = wtile("w2", w2_d, [128, FCH, E], "(c p) n -> p c n", eng=nc.scalar)
        b2r = wtile("b2", b2_d, [1, E], eng=nc.scalar)
        # f_b1 (+ folded ln3_b @ w1) in column layout for the relu bias
        b1c = wpool.tile([128, FCH], F32, tag="b1c", bufs=1, name="b1c")
        b1cb = wpool.tile([128, FCH], BF16, tag="b1cb", bufs=1, name="b1cb")
        nc.scalar.dma_start(b1cb[...],
                            b1_d[:].rearrange("o (c p) -> p (o c)", p=128))
        nc.vector.tensor_copy(b1c[:, :], b1cb[:, :])
        eps = wpool.tile([128, 1], F32, tag="eps", bufs=1, name="eps")
        nc.gpsimd.memset(eps[:, :], 1e-5)

        # ---- per-pair stage closures, emitted in a software-pipelined
        # wavefront so another pair's PE work sits next to every LN chain ----
        st = {}  # pr -> {'x_el', 'pm2', 'sm2'}

        def stage_load(pr):
            els = (2 * pr, 2 * pr + 1)
            x_el = []
            pm2 = pools["mbc"].tile([128, 2 * T], BF16, name="pm2")
            sm2 = pools["mbc"].tile([128, 2 * T], BF16, name="sm2")
            for el, e in enumerate(els):
                x_cs = []
                for (t0, tc) in TCH:
                    xt = pools["res"].tile([tc, E], F32, name="x_in",
                                           tag="res")
                    nc.sync.dma_start(xt[:, :], x_d[e, t0:t0 + tc, :])
                    x_cs.append(xt)
                x_el.append(x_cs)
                pm_row = pools["mrow"].tile([1, T], BF16, name="pm_row", bufs=2)
                nc.sync.dma_start(pm_row[:, :], pm_d[e:e + 1, :])
                nc.gpsimd.partition_broadcast(pm2[:, el * T:(el + 1) * T],
                                              pm_row[:, :])
                sm_row = pools["mrow"].tile([1, T], BF16, name="sm_row", bufs=2)
                nc.sync.dma_start(sm_row[:, :], sm_d[e:e + 1, :])
                nc.gpsimd.partition_broadcast(sm2[:, el * T:(el + 1) * T],
                                              sm_row[:, :])
            st[pr] = {"x_el": x_el, "pm2": pm2, "sm2": sm2}

        def stage_sa(pr):
            s = st[pr]
            x_el = s["x_el"]
            h_pair = [[_layernorm(nc, pools, x_el[el][ci][:, :], tc, eps)
                       for ci, (t0, tc) in enumerate(TCH)] for el in range(2)]
            hT = _transpose_pair(nc, pools, h_pair, identb)
            qT = _project_qkT(nc, pools, wq_sa, hT, "q_sa", bq_sa, s["pm2"])
            kT = _project_qkT(nc, pools, wk_sa, hT, "k_sa", bk_sa, s["pm2"])
            for el in range(2):
                v_sb = _project_v(nc, pools, wv_sa, hT, el * T, "v_sa")
                x_el[el] = _attention(nc, pools, (qT, kT), v_sb, ones64,
                                      wo_sa, bo_sa, ones_row, x_el[el], el * T)

        def stage_ca(pr):
            s = st[pr]
            x_el = s["x_el"]
            els = (2 * pr, 2 * pr + 1)
            h_pair = [[_layernorm(nc, pools, x_el[el][ci][:, :], tc, eps)
                       for ci, (t0, tc) in enumerate(TCH)] for el in range(2)]
            h2T = _transpose_pair(nc, pools, h_pair, identb)
            m_pair = []
            for el, e in enumerate(els):
                m_cs = []
                for (t0, tc) in TCH:
                    mt = pools["h"].tile([tc, E], BF16, name="m_nat",
                                         tag="m_nat", bufs=6)
                    nc.sync.dma_start(mt[:, :], mem_d[e, t0:t0 + tc, :])
                    m_cs.append(mt)
                m_pair.append(m_cs)
            mT = _transpose_pair(nc, pools, m_pair, identb)
            qT = _project_qkT(nc, pools, wq_ca, h2T, "q_ca", bq_ca)
            kT = _project_qkT(nc, pools, wk_ca, mT, "k_ca", mask=s["sm2"])
            for el in range(2):
                v_sb = _project_v(nc, pools, wv_ca, h2T, el * T, "v_ca")
                x_el[el] = _attention(nc, pools, (qT, kT), v_sb, ones64,
                                      wo_ca, bo_ca, ones_row, x_el[el], el * T)

        def stage_ffn(pr):
            s = st[pr]
            x_el = s["x_el"]
            els = (2 * pr, 2 * pr + 1)
            h_pair = [[_layernorm(nc, pools, x_el[el][ci][:, :], tc, eps)
                       for ci, (t0, tc) in enumerate(TCH)] for el in range(2)]
            h3T = _transpose_pair(nc, pools, h_pair, identb)
            rT = []
            for fc in range(FCH):
                zps = pools["ps"].tile([128, 2 * T], F32, name="z_ps",
                                       tag="ps")
                for ec in range(ECH):
                    nc.tensor.matmul(zps[:, :],
                                     w1[:, ec, fc * 128:(fc + 1) * 128],
                                     h3T[ec][:, :], start=(ec == 0),
                                     stop=(ec == 3))
                r = pools["rT"].tile([128, 2 * T], BF16, name="r")
                nc.scalar.activation(r[:, :], zps[:, :], AF.Relu,
                                     bias=b1c[:, fc:fc + 1])
                rT.append(r)
            for el, e in enumerate(els):
                for ci, (t0, tc) in enumerate(TCH):
                    yps = pools["ps"].tile([tc, E], F32, name="y_ps",
                                           tag="ps")
                    for fc in range(FCH):
                        nc.tensor.matmul(yps[:, :],
                                         rT[fc][:, el * T + t0:el * T + t0 + tc],
                                         w2[:, fc, :], start=(fc == 0),
                                         stop=False)
                    nc.tensor.matmul(yps[:, :], ones_row[0:1, 0:tc],
                                     b2r[0:1, :], start=False, stop=True)
                    yout = pools["res"].tile([tc, E], F32, name="yout",
                                             tag="res")
                    nc.vector.tensor_add(yout[:, :], yps[:, :],
                                         x_el[el][ci][:, :])
                    nc.sync.dma_start(out_d[e, t0:t0 + tc, :], yout[:, :])
            del st[pr]

        npr = bpc // 2
        stage_fns = [stage_sa, stage_ca, stage_ffn]
        stage_load(0)
        for step in range(npr + 2):
            if step + 1 < npr:
                stage_load(step + 1)
            # deeper stages of older pairs first, then the fresh pair's SA
            for sidx in range(2, -1, -1):
                pr = step - sidx
                if 0 <= pr < npr:
                    stage_fns[sidx](pr)

    nc.compile()
    return nc


def _host_prep(inputs, bpc, core):
    """Build the in_map for one core."""
    s = slice(core * bpc, (core + 1) * bpc)

    def rearr(w, g=None):  # (H, E, D) -> [E, H*D], optionally row-scaled
        m = np.transpose(np.asarray(w, np.float32), (1, 0, 2)).reshape(E, E)
        if g is not None:
            m = m * np.asarray(g, np.float32)[:, None]
        return np.ascontiguousarray(m).astype(NPBF16)

    def b16(a):
        return np.ascontiguousarray(np.asarray(a, np.float32)).astype(NPBF16)

    def f32c(a):
        return np.ascontiguousarray(np.asarray(a, np.float32))

    g1 = np.asarray(inputs["ln1_g"], np.float32)
    b1n = np.asarray(inputs["ln1_b"], np.float32)
    g2 = np.asarray(inputs["ln2_g"], np.float32)
    b2n = np.asarray(inputs["ln2_b"], np.float32)
    g3 = np.asarray(inputs["ln3_g"], np.float32)
    b3n = np.asarray(inputs["ln3_b"], np.float32)

    def wr(w):  # raw rearranged fp32 (for beta @ W rows)
        return np.transpose(np.asarray(w, np.float32), (1, 0, 2)).reshape(E, E)

    def bcolT(b):  # [E] bias row -> [128, 4] per-partition columns per oc
        return np.ascontiguousarray(
            np.asarray(b, np.float32).reshape(4, 128).T)

    wo_sa_f = np.asarray(inputs["sa_wo"], np.float32)
    wo_ca_f = np.asarray(inputs["ca_wo"], np.float32)
    bv_sa = b1n @ wr(inputs["sa_wv"])
    bv_ca = b2n @ wr(inputs["ca_wv"])

    return {
        "x": f32c(inputs["idx"][s]),
        "mem": b16(inputs["memory"][s]),
        "pm": b16(inputs["pred_mask"][s] != 0),
        "sm": b16(inputs["src_mask"][s] != 0),
        "wq_sa": rearr(inputs["sa_wq"], g1), "wk_sa": rearr(inputs["sa_wk"], g1),
        "wv_sa": rearr(inputs["sa_wv"], g1),
        "wo_sa": b16(inputs["sa_wo"]),
        "bo_sa": b16(np.asarray(inputs["sa_bo"], np.float32)
                     + bv_sa @ wo_sa_f).reshape(1, E),
        "bq_sa": bcolT(b1n @ wr(inputs["sa_wq"])),
        "bk_sa": bcolT(b1n @ wr(inputs["sa_wk"])),
        "wq_ca": rearr(inputs["ca_wq"], g2), "wk_ca": rearr(inputs["ca_wk"]),
        "wv_ca": rearr(inputs["ca_wv"], g2),
        "wo_ca": b16(inputs["ca_wo"]),
        "bo_ca": b16(np.asarray(inputs["ca_bo"], np.float32)
                     + bv_ca @ wo_ca_f).reshape(1, E),
        "bq_ca": bcolT(b2n @ wr(inputs["ca_wq"])),
        "w1": b16(np.asarray(inputs["f_w1"], np.float32)
                  * g3[:, None]),
        "b1": b16(np.asarray(inputs["f_b1"], np.float32)
                  + b3n @ np.asarray(inputs["f_w1"], np.float32)).reshape(1, F),
        "w2": b16(inputs["f_w2"]), "b2": b16(inputs["f_b2"]).reshape(1, E),
    }


def get_program(bpc):
    if bpc not in _programs:
        _programs[bpc] = _build(bpc)
    return _programs[bpc]


def kernel(**inputs) -> np.ndarray:
    bpc = B // NCORES
    nc = get_program(bpc)
    in_maps = [_host_prep(inputs, bpc, c) for c in range(NCORES)]
    res = run_bass_kernel_spmd(nc, in_maps, core_ids=list(range(NCORES)))
    out = np.concatenate([res.results[c]["out"] for c in range(NCORES)], axis=0)
    return out.astype(np.float32)

